# revision 40
# baseline (speedup 1.0000x reference)
"""Trainium2 Bass kernel for the EnhancedEncoderLayer (dense MHA + low-rank
top-k sparse attention + FFN, two layernorms).

Sharding: 8 cores = (batch b in 0..3) x (query-half h in {0,1}). Each core
computes output rows [b, h*512:(h+1)*512, :]. K/V-side projections are
computed redundantly per batch pair (no cross-core communication).

The host permutes src[b].T columns so each core's own query tokens are
columns 0..511 (attention contracts over all keys, so key order is
irrelevant); this keeps the SPMD program identical across cores.

Precision: all projections run bf16 x bf16 (PSUM accumulation is fp32);
the low-rank sparse-score matmul stays f32r. psp/exp values, the spmm and
the LN2 residual are bf16. Measured rel err ~4.3e-3 vs the 2e-2 gate.

Host-side folds (exact math): projection biases of the v/vsp paths commute
through the (normalized) attention rows into one residual bias row;
sigmoid(lam) is precomputed into the column-bias pack; ln1's affine is
folded into the ff1 weights; all row-broadcast constants are shipped
pre-replicated (no on-device partition broadcasts on the critical path).

Schedule highlights:
- per-head-pair score matmuls target PE row-groups 0:64/64:128 and overlap;
  both land in one 2-bank PSUM tile so a single N=1024 exp evacuates them;
- v/vsp projections computed token-major (stationary = xT tile) - no PE
  transposes; k/q proj PSUM evacuation on DVE, off the exp-saturated ACT;
- the 10-iter top-k threshold bisection (DVE) overlaps dense-path PE work;
- out_proj + spmm + fuse + LN1 run as a per-query-tile pipeline; the
  LN1 output transpose for ff1 uses XBAR DMA-transposes (no PE/DVE cost);
- ff2 streams in two column-half passes; the xg residual is folded into
  PSUM via an identity matmul and LN2 finishes per qt inside the second
  pass, overlapping the tail.
"""
import sys
import os
import contextlib

for _p in ('/opt/trn_rl_repo',):
    if _p not in sys.path:
        sys.path.insert(0, _p)

import numpy as np
import concourse.bacc as bacc
import concourse.tile as tile
from concourse import mybir
from concourse.bass_utils import run_bass_kernel_spmd
from concourse.masks import make_identity

F32 = mybir.dt.float32
F32R = mybir.dt.float32r
BF16 = mybir.dt.bfloat16
AF = mybir.ActivationFunctionType
OP = mybir.AluOpType

B, S, D, H, R, DFF = 4, 1024, 1024, 16, 64, 4096
DH = D // H          # 64
SQ = S // 2          # 512 own queries per core
KK = max(1, int(S * 0.2))   # 204
KC = D // 128        # 8 contraction chunks over D
FC = DFF // 128      # 32 chunks over DFF
NQT = SQ // 128      # 4 query tiles
NTOK = S // 128      # 8 token tiles
BISECT_ITERS = 10
INV_SQRT = 0.125     # 1/sqrt(DH) == 1/sqrt(R)

_cached = {}


def _build():
    nc = bacc.Bacc()

    def din(name, shape):
        return nc.declare_dram_parameter(name, list(shape), F32, isOutput=False)

    xT = nc.declare_dram_parameter("xT", [D, S], BF16, isOutput=False)
    x_own = din("x_own", [SQ, D])   # own rows, token-major (f32 residual)
    wqkvT = nc.declare_dram_parameter("wqkvT", [D, 3 * D], BF16,
                                      isOutput=False)
    woT = nc.declare_dram_parameter("woT", [D, D], BF16, isOutput=False)
    vpT = nc.declare_dram_parameter("vpT", [D, D], BF16, isOutput=False)
    qkpT = nc.declare_dram_parameter("qkpT", [D, 2 * R], BF16,
                                     isOutput=False)
    f1T = nc.declare_dram_parameter("f1T", [D, DFF], BF16, isOutput=False)
    f2T = nc.declare_dram_parameter("f2T", [DFF, D], BF16, isOutput=False)
    # column-bias pack (host-side layout, one contiguous DMA):
    # [:, 0:24]=in_proj_b  [:, 24:32]=Vp_b  [:, 32:64]=ff1_b
    # [:, 64:72]=ln1_g  [:, 72:80]=ln1_b  [0:64, 80]=Qp_b  [0:64, 81]=Kp_b
    bias_pack = din("bias_pack", [128, 84])
    # host-replicated row constants: 0=residual bias row, 1=b2+ln1_b,
    # 2=ln1_g, 3=ln2_g, 4=ln2_b
    rows_pack = din("rows_pack", [128, 5 * D])
    out = nc.declare_dram_parameter("out", [SQ, D], F32, isOutput=True)
    DBG = bool(os.environ.get("BASSK_DEBUG"))
    if DBG:
        dbg_dense = nc.declare_dram_parameter("dbg_dense", [SQ, D], F32,
                                              isOutput=True)
        dbg_sparse = nc.declare_dram_parameter("dbg_sparse", [SQ, D], F32,
                                               isOutput=True)
        dbg_lo = nc.declare_dram_parameter("dbg_lo", [128, NQT], F32,
                                           isOutput=True)
        dbg_rs = nc.declare_dram_parameter("dbg_rs", [128, NQT], F32,
                                           isOutput=True)

    xT_r = xT.ap().rearrange("(kc p) s -> p kc s", p=128)
    wqkvT_r = wqkvT.ap().rearrange("(kc p) f -> p kc f", p=128)
    woT_r = woT.ap().rearrange("(kc p) f -> p kc f", p=128)
    vpT_r = vpT.ap().rearrange("(kc p) f -> p kc f", p=128)
    qkpT_r = qkpT.ap().rearrange("(kc p) f -> p kc f", p=128)
    f1T_r = f1T.ap().rearrange("(kc p) f -> p kc f", p=128)
    f2T_r = f2T.ap().rearrange("(kc p) f -> p kc f", p=128)

    with tile.TileContext(nc) as tc:
        est = contextlib.ExitStack()
        with est:
            # ---------------- constants ----------------
            consts = est.enter_context(tc.tile_pool(name="consts", bufs=1))

            ident_f = consts.tile([128, 128], F32, name="ident_f")
            make_identity(nc, ident_f)
            ident_b = consts.tile([128, 128], BF16, name="ident_b")
            nc.vector.tensor_copy(out=ident_b, in_=ident_f)

            eps_t = consts.tile([128, 1], F32, name="eps_t")
            nc.vector.memset(eps_t, 1e-5)
            ones1 = consts.tile([128, 1], F32, name="ones1")
            nc.vector.memset(ones1, 1.0)
            ones16 = consts.tile([128, 16], F32, name="ones16")
            nc.vector.memset(ones16, 1.0)

            bp = consts.tile([128, 84], F32, name="bias_pack")
            bqkv_c = bp[:, 0:24]
            bvp_c = bp[:, 24:32]
            b1_c = bp[:, 32:64]
            g1_c = bp[:, 64:72]
            be1_c = bp[:, 72:80]
            bqp_c = bp[0:64, 80:81]
            bkp_c = bp[0:64, 81:82]
            sig_bc = bp[:, 82:83]
            oms_bc = bp[:, 83:84]
            ones1b = consts.tile([128, 1], BF16, name="ones1b")
            nc.vector.memset(ones1b, 1.0)
            sd_pre = consts.tile([1, 1], F32, name="sd_pre")

            def load_bias_cols():
                # one contiguous transfer on the (otherwise idle) Pool ring
                nc.gpsimd.dma_start(out=bp, in_=bias_pack.ap())


            # own-token residual (+ sig*bo); loaded after xT is in flight
            xot_pool = est.enter_context(tc.tile_pool(name="xot_pool",
                                                      bufs=1))
            xot = xot_pool.tile([128, NQT, D], F32, name="xot")

            bis = est.enter_context(tc.tile_pool(name="bis", bufs=1))
            lo = bis.tile([128, NQT], F32, name="lo")
            hi = bis.tile([128, NQT], F32, name="hi")
            mid = bis.tile([128, NQT], F32, name="mid")
            cnts = bis.tile([128, NQT], F32, name="cnts")
            pred = bis.tile([128, NQT], mybir.dt.uint32, name="pred")
            rs_sp = bis.tile([128, NQT], F32, name="rs_sp")
            rcp_sp = bis.tile([128, NQT], F32, name="rcp_sp")

            # long-lived activation groups (left stack)
            sp_stack = contextlib.ExitStack()
            sp_pool = sp_stack.enter_context(
                tc.tile_pool(name="sp_pool", bufs=1))
            Vsp = sp_pool.tile([128, NTOK, D], BF16, name="Vsp")
            kspT = sp_pool.tile([64, S], F32R, name="kspT")
            qspT = sp_pool.tile([64, SQ], F32R, name="qspT")

            wo_full = sp_pool.tile([128, KC, D], BF16, name="wo_full")

            dn_stack = contextlib.ExitStack()
            dn_pool = dn_stack.enter_context(
                tc.tile_pool(name="dn_pool", bufs=1))
            kT = dn_pool.tile([128, KC, S], BF16, name="kT")
            Vaug = dn_pool.tile([128, NTOK, H * (DH + 1)], BF16, name="Vaug")
            qT = dn_pool.tile([128, KC, SQ], BF16, name="qT")

            Vaug_h = Vaug.rearrange("p t (h c) -> p t h c", c=DH + 1)
            for t in range(NTOK):
                nc.vector.tensor_copy(out=Vaug_h[:, t, :, DH:DH + 1],
                                      in_=ones16)

            # right-stack pools (all close together after phase 7)
            psp_stack = contextlib.ExitStack()
            psp_pool = psp_stack.enter_context(
                tc.tile_pool(name="psp_pool", bufs=1, side="right"))
            psp = [psp_pool.tile([128, S], BF16, name=f"psp{qt}")
                   for qt in range(NQT)]
            scr_stack = contextlib.ExitStack()
            scr_pool = scr_stack.enter_context(
                tc.tile_pool(name="scr", bufs=1, side="right"))
            ctx_stack = contextlib.ExitStack()
            ctx_pool = ctx_stack.enter_context(
                tc.tile_pool(name="ctx_pool", bufs=1, side="right"))
            ctxT = ctx_pool.tile([128, KC, SQ], BF16, name="ctxT")
            # ============ projections + sparse path + attention ============
            with contextlib.ExitStack() as ph0:
                xt_pool = ph0.enter_context(
                    tc.tile_pool(name="xt_pool", bufs=1))
                wstr = ph0.enter_context(tc.tile_pool(name="wstr", bufs=8))
                pt_pool = ph0.enter_context(
                    tc.tile_pool(name="pt_pool", bufs=4))
                rc_pool = ph0.enter_context(
                    tc.tile_pool(name="rc_pool", bufs=1))
                ps_a = ph0.enter_context(
                    tc.tile_pool(name="ps_a", bufs=2, space="PSUM"))
                ps_b = ph0.enter_context(
                    tc.tile_pool(name="ps_b", bufs=4, space="PSUM"))

                # small sparse weights first, then xT on both queues
                qkpt = wstr.tile([128, KC, 2 * R], BF16, name="qkpt",
                                 tag="wsmall")
                nc.sync.dma_start(out=qkpt, in_=qkpT_r)
                qpt = qkpt[:, :, 0:R]
                kpt = qkpt[:, :, R:2 * R]
                xTt = xt_pool.tile([128, KC, S], BF16, name="xTt")
                for kc2 in range(4):
                    eng = nc.scalar if kc2 % 2 == 0 else nc.sync
                    eng.dma_start(out=xTt[:, 2 * kc2:2 * kc2 + 2, :],
                                  in_=xT_r[:, 2 * kc2:2 * kc2 + 2, :])
                load_bias_cols()
                vw_stack = contextlib.ExitStack()
                vw_pool = vw_stack.enter_context(
                    tc.tile_pool(name="vw_pool", bufs=2))

                # ---- sparse projections + scores ----
                with nc.named_scope("p0_ksp_qsp"):
                    ka = ps_a.tile([128, 1024], F32, name="ksa", tag="psa")
                    for nh in range(2):
                        for kc in range(KC):
                            nc.tensor.matmul(
                                ka[0:64, nh * 512:nh * 512 + 512],
                                kpt[:, kc, :],
                                xTt[:, kc, nh * 512:nh * 512 + 512],
                                start=(kc == 0), stop=(kc == KC - 1))
                    nc.scalar.activation(
                        out=kspT, in_=ka[0:64, :], func=AF.Identity,
                        bias=bkp_c, scale=1.0)
                    ps = ps_b.tile([128, 512], F32, name="ps", tag="psb")
                    for kc in range(KC):
                        nc.tensor.matmul(ps[0:64, :], qpt[:, kc, :],
                                         xTt[:, kc, 0:SQ],
                                         start=(kc == 0), stop=(kc == KC - 1))
                    nc.scalar.activation(out=qspT, in_=ps[0:64, :],
                                         func=AF.Identity, bias=bqp_c,
                                         scale=1.0)

                with nc.named_scope("p2_ssp"):
                    for qt in range(NQT):
                        ps2 = ps_a.tile([128, 1024], F32, name="ps2",
                                        tag="psa")
                        for nh in range(2):
                            nc.tensor.matmul(
                                ps2[:, nh * 512:nh * 512 + 512],
                                qspT[:, qt * 128:qt * 128 + 128],
                                kspT[:, nh * 512:nh * 512 + 512],
                                start=True, stop=True)
                        nc.scalar.activation(
                            out=psp[qt], in_=ps2, func=AF.Exp,
                            scale=INV_SQRT)

                # ---- top-k threshold bisection (DVE; overlaps PE below) ----
                with nc.named_scope("p3_bisect"):
                    nc.vector.memset(lo, 0.0)
                    nc.vector.memset(hi, 16.0)
                    for it in range(BISECT_ITERS):
                        nc.vector.tensor_add(mid, lo, hi)
                        nc.vector.tensor_scalar_mul(mid, mid, 0.5)
                        for qt in range(NQT):
                            scr = scr_pool.tile([128, S], BF16, name="scr",
                                                tag="scr")
                            nc.vector.scalar_tensor_tensor(
                                out=scr, in0=psp[qt],
                                scalar=mid[:, qt:qt + 1],
                                in1=ones1b.to_broadcast([128, S]),
                                op0=OP.is_ge, op1=OP.mult,
                                accum_out=cnts[:, qt:qt + 1])
                        nc.vector.tensor_scalar(out=pred, in0=cnts,
                                                scalar1=float(KK),
                                                scalar2=None, op0=OP.is_ge)
                        nc.vector.copy_predicated(lo, pred, mid)
                        nc.vector.tensor_scalar(out=pred, in0=cnts,
                                                scalar1=float(KK),
                                                scalar2=None, op0=OP.is_lt)
                        nc.vector.copy_predicated(hi, pred, mid)
                    for qt in range(NQT):
                        nc.vector.scalar_tensor_tensor(
                            out=psp[qt], in0=psp[qt],
                            scalar=lo[:, qt:qt + 1],
                            in1=psp[qt], op0=OP.is_ge, op1=OP.mult,
                            accum_out=rs_sp[:, qt:qt + 1])
                    if DBG:
                        nc.sync.dma_start(out=dbg_lo.ap(), in_=lo)
                        nc.sync.dma_start(out=dbg_rs.ap(), in_=rs_sp)
                    nc.vector.tensor_scalar(out=rs_sp, in0=rs_sp,
                                            scalar1=1e-9, scalar2=None,
                                            op0=OP.add)
                    nc.vector.reciprocal(rcp_sp, rs_sp)
                    nc.vector.tensor_scalar_mul(rcp_sp, rcp_sp, oms_bc)

                for qt in range(NQT):
                    nc.gpsimd.dma_start(
                        out=xot[:, qt, :],
                        in_=x_own.ap()[qt * 128:qt * 128 + 128, :])

                _wc_cnt = [0]

                def w_chunk(w_view, f0, nfs=128):
                    wt = wstr.tile([128, KC, 128], BF16, name="wt", tag="wt")
                    eng = nc.sync if _wc_cnt[0] % 2 == 0 else nc.scalar
                    _wc_cnt[0] += 1
                    eng.dma_start(out=wt[:, :, :nfs],
                                  in_=w_view[:, :, f0:f0 + nfs])
                    return wt

                # ---- v / vsp projections, directly token-major ----
                # stationary = xT token-tile chunk, moving = weight rows, so
                # the PSUM result lands token-major (no transposes, no
                # copies). Projection biases are folded into the residual
                # row host-side (softmax rows sum to 1).
                def proj_tokmajor(w_view, f_lo, to_vaug, scope):
                    with nc.named_scope(scope):
                        for fh in range(2):
                            wh = vw_pool.tile([128, KC, 512], BF16,
                                              name="wh", tag="wh")
                            eng = nc.scalar if fh == 0 else nc.sync
                            eng.dma_start(
                                out=wh,
                                in_=w_view[:, :,
                                           f_lo + fh * 512:
                                           f_lo + fh * 512 + 512])
                            for t in range(NTOK):
                                ps = ps_b.tile([128, 512], F32, name="psv",
                                               tag="psb")
                                for kc in range(KC):
                                    nc.tensor.matmul(
                                        ps,
                                        xTt[:, kc, t * 128:t * 128 + 128],
                                        wh[:, kc, :],
                                        start=(kc == 0), stop=(kc == KC - 1))
                                if to_vaug:
                                    nc.scalar.activation(
                                        out=Vaug_h[:, t, 8 * fh:8 * fh + 8,
                                                   0:DH],
                                        in_=ps, func=AF.Identity, scale=1.0)
                                else:
                                    nc.scalar.activation(
                                        out=Vsp[:, t,
                                                fh * 512:fh * 512 + 512],
                                        in_=ps, func=AF.Identity, scale=1.0)

                proj_tokmajor(wqkvT_r, 2 * D, True, "p0_v")
                proj_tokmajor(vpT_r, 0, False, "p0_vsp")
                vw_stack.close()

                # out_proj weights (needed only after attention)
                nc.scalar.dma_start(out=wo_full, in_=woT_r)

                # ---- interleaved k/q projections + dense attention ----
                with nc.named_scope("p4_kq_attn"):
                    for jj in range(4):
                        for fi in range(2):
                            ft = jj * 2 + fi
                            wkc = w_chunk(wqkvT_r, D + ft * 128)
                            for nh in range(2):
                                ps = ps_b.tile([128, 512], F32, name="ps",
                                               tag="psb")
                                for kc in range(KC):
                                    nc.tensor.matmul(
                                        ps,
                                        wkc[:, kc, 0:128],
                                        xTt[:, kc, nh * 512:nh * 512 + 512],
                                        start=(kc == 0), stop=(kc == KC - 1))
                                nc.vector.tensor_scalar(
                                    out=kT[:, ft, nh * 512:nh * 512 + 512],
                                    in0=ps,
                                    scalar1=bqkv_c[:, 8 + ft:8 + ft + 1],
                                    scalar2=None, op0=OP.add)
                            wqc = w_chunk(wqkvT_r, ft * 128)
                            ps = ps_b.tile([128, 512], F32, name="ps",
                                           tag="psb")
                            for kc in range(KC):
                                nc.tensor.matmul(
                                    ps, wqc[:, kc, 0:128],
                                    xTt[:, kc, 0:SQ],
                                    start=(kc == 0), stop=(kc == KC - 1))
                            nc.vector.tensor_scalar(
                                out=qT[:, ft, :], in0=ps,
                                scalar1=bqkv_c[:, ft:ft + 1],
                                scalar2=None, op0=OP.add)
                        # attention for the 4 heads of these two f-tiles.
                        # The two heads of an f-tile sit on partition rows
                        # 0:64 / 64:128, so their score matmuls target
                        # different PE row-groups and overlap when issued
                        # back-to-back; both land in one 2-bank PSUM tile so
                        # a single N=1024 exp evacuates the pair.
                        for ft in (2 * jj, 2 * jj + 1):
                            pctx = {po: ps_b.tile([128, 512], F32,
                                                  name="ps_c", tag="psb")
                                    for po in (0, 64)}
                            for t in range(NTOK):
                                ps2 = ps_a.tile([128, 1024], F32,
                                                name="ps_s", tag="psa")
                                for po in (0, 64):
                                    nc.tensor.matmul(
                                        ps2[:, 8 * po:8 * po + 512],
                                        kT[po:po + 64, ft,
                                           t * 128:t * 128 + 128],
                                        qT[po:po + 64, ft, :],
                                        start=True, stop=True)
                                pt = pt_pool.tile([128, 1024], BF16,
                                                  name="pT", tag="pT")
                                nc.scalar.activation(out=pt, in_=ps2,
                                                     func=AF.Exp,
                                                     scale=INV_SQRT)
                                for po in (0, 64):
                                    hh = 2 * ft + po // 64
                                    nc.tensor.matmul(
                                        pctx[po][0:65, :],
                                        Vaug[:, t, hh * 65:hh * 65 + 65],
                                        pt[:, 8 * po:8 * po + 512],
                                        start=(t == 0),
                                        stop=(t == NTOK - 1))
                            for po in (0, 64):
                                rsr = rc_pool.tile([1, 512], F32, name="rsr",
                                                   tag="rsr")
                                nc.vector.tensor_copy(out=rsr,
                                                      in_=pctx[po][64:65, :])
                                rch = rc_pool.tile([1, 512], F32, name="rch",
                                                   tag="rch")
                                nc.vector.reciprocal_approx_fast(out=rch,
                                                                 in_=rsr)
                                rb = rc_pool.tile([64, 512], F32, name="rb",
                                                  tag="rb")
                                nc.gpsimd.partition_broadcast(rb, rch)
                                nc.vector.tensor_mul(
                                    out=ctxT[po:po + 64, ft, :],
                                    in0=pctx[po][0:64, :], in1=rb)

                # pull the exp->sqrt ACT table switch off the LN1 chain
                nc.scalar.activation(out=sd_pre, in_=eps_t[0:1, :],
                                     func=AF.Sqrt, bias=eps_t[0:1, :],
                                     scale=1.0)

            dn_stack.close()   # free kT, Vaug, qT

            ds_stack = contextlib.ExitStack()
            ds_pool = ds_stack.enter_context(
                tc.tile_pool(name="ds_pool", bufs=1, side="right"))
            dense_s = ds_pool.tile([128, NQT, D], F32, name="dense_s")
            sparse_s = ds_pool.tile([128, NQT, D], F32, name="sparse_s")

            # ---- host-replicated row constants (one DMA on the idle
            # sync ring; lands mid-attention) ----
            rows_t = ds_pool.tile([128, 5, D], F32, name="rows_t")
            nc.sync.dma_start(
                out=rows_t,
                in_=rows_pack.ap().rearrange("p (r d) -> p r d", r=5))
            b12_bc = rows_t[:, 1, :]
            g1_bc = rows_t[:, 2, :]
            g2_bc = rows_t[:, 3, :]
            be2_bc = rows_t[:, 4, :]

            xg = ds_pool.tile([128, NQT, D], BF16, name="xg")
            stats = ds_pool.tile([128, NQT, 2, 6], F32, name="stats")
            mv2 = ds_pool.tile([128, NQT, 2], F32, name="mv2")
            sd = ds_pool.tile([128, NQT], F32, name="sd")
            rstd = ds_pool.tile([128, NQT], F32, name="rstd")
            x1s = ds_pool.tile([128, 2, D], F32, name="x1s")
            xhat_bf = ds_pool.tile([128, NQT, D], BF16, name="xhat_bf")

            def ln_normalize(x1, qt, out=None):
                for half in range(2):
                    nc.vector.bn_stats(
                        out=stats[:, qt, half, :],
                        in_=x1[:, half * 512:half * 512 + 512])
                nc.vector.bn_aggr(out=mv2[:, qt, :], in_=stats[:, qt])
                nc.scalar.activation(out=sd[:, qt:qt + 1],
                                     in_=mv2[:, qt, 1:2], func=AF.Sqrt,
                                     bias=eps_t, scale=1.0)
                nc.vector.reciprocal(rstd[:, qt:qt + 1], sd[:, qt:qt + 1])
                nc.vector.tensor_scalar(out=x1 if out is None else out,
                                        in0=x1,
                                        scalar1=mv2[:, qt, 0:1],
                                        scalar2=rstd[:, qt:qt + 1],
                                        op0=OP.subtract, op1=OP.mult)

            xln_stack = contextlib.ExitStack()
            xln_pool = xln_stack.enter_context(
                tc.tile_pool(name="xln_pool", bufs=1, side="right"))
            xlnT = xln_pool.tile([128, KC, SQ], BF16, name="xlnT")

            # ====== per-qt pipeline: out_proj + spmm + fuse + LN1 + xT ======
            with contextlib.ExitStack() as ph5:
                pm_pool = ph5.enter_context(
                    tc.tile_pool(name="pm_pool", bufs=1))
                ps_tr2 = ph5.enter_context(
                    tc.tile_pool(name="ps_tr2", bufs=4, space="PSUM"))
                ps_mm = ph5.enter_context(
                    tc.tile_pool(name="ps_mm", bufs=4, space="PSUM"))
                pmT = pm_pool.tile([128, NTOK, SQ], BF16, name="pmT")
                with nc.named_scope("p5_outproj"):
                    # all masked-p transposes first: they depend only on
                    # psp, so the PE stays busy while the attention tail's
                    # DVE normalize chain drains
                    for qt in range(NQT):
                        for t in range(NTOK):
                            pst = ps_tr2.tile([128, 128], BF16, name="pst2",
                                              tag="pst2")
                            nc.tensor.transpose(
                                pst, psp[qt][:, t * 128:t * 128 + 128],
                                ident_b)
                            nc.scalar.copy(
                                out=pmT[:, t, qt * 128:qt * 128 + 128],
                                in_=pst)
                    for qt in range(NQT):
                        for nh in range(2):
                            ps = ps_mm.tile([128, 512], F32, name="ps_o",
                                            tag="ps_o")
                            for t in range(NTOK):
                                nc.tensor.matmul(
                                    ps,
                                    pmT[:, t, qt * 128:qt * 128 + 128],
                                    Vsp[:, t, nh * 512:nh * 512 + 512],
                                    start=(t == 0), stop=(t == NTOK - 1))
                            nc.scalar.activation(
                                out=sparse_s[:, qt, nh * 512:nh * 512 + 512],
                                in_=ps, func=AF.Copy,
                                scale=rcp_sp[:, qt:qt + 1])
                        # dense out_proj for this query tile
                        pss2 = [ps_mm.tile([128, 512], F32, name="ps_o",
                                           tag="ps_o") for _ in range(2)]
                        for kc in range(KC):
                            for nh in range(2):
                                nc.tensor.matmul(
                                    pss2[nh],
                                    ctxT[:, kc, qt * 128:qt * 128 + 128],
                                    wo_full[:, kc, nh * 512:nh * 512 + 512],
                                    start=(kc == 0), stop=(kc == KC - 1))
                        for nh in range(2):
                            nc.scalar.activation(
                                out=dense_s[:, qt, nh * 512:nh * 512 + 512],
                                in_=pss2[nh], func=AF.Copy, scale=sig_bc)
                        # fuse + LN1 + transpose for ff1
                        x1 = x1s[:, qt % 2, :]
                        nc.vector.tensor_add(x1[:, 0:512],
                                             dense_s[:, qt, 0:512],
                                             sparse_s[:, qt, 0:512])
                        nc.gpsimd.tensor_add(x1[:, 512:1024],
                                             dense_s[:, qt, 512:1024],
                                             sparse_s[:, qt, 512:1024])
                        nc.vector.tensor_add(x1[:, 0:512], x1[:, 0:512],
                                             xot[:, qt, 0:512])
                        nc.gpsimd.tensor_add(x1[:, 512:1024],
                                             x1[:, 512:1024],
                                             xot[:, qt, 512:1024])
                        ln_normalize(x1, qt, out=xhat_bf[:, qt, :])
                        nc.sync.dma_start_transpose(
                            out=xlnT[:, :, qt * 128:qt * 128 + 128],
                            in_=xhat_bf[:, qt, :])
            if DBG:
                for qt in range(NQT):
                    nc.sync.dma_start(
                        out=dbg_dense.ap()[qt * 128:qt * 128 + 128, :],
                        in_=dense_s[:, qt, :])
                    nc.sync.dma_start(
                        out=dbg_sparse.ap()[qt * 128:qt * 128 + 128, :],
                        in_=sparse_s[:, qt, :])
            sp_stack.close()

            # xg (LN2 residual) on DVE while ff1 owns the PE
            for qt in range(NQT):
                nc.vector.tensor_mul(xg[:, qt, :], xhat_bf[:, qt, :], g1_bc)
                nc.vector.tensor_add(xg[:, qt, :], xg[:, qt, :], b12_bc)

            # ============ ff1 + relu ============
            h1_stack = contextlib.ExitStack()
            h1_pool = h1_stack.enter_context(
                tc.tile_pool(name="h1_pool", bufs=1))
            h1T = h1_pool.tile([128, FC, SQ], BF16, name="h1T")
            with contextlib.ExitStack() as ph9:
                w3str = ph9.enter_context(tc.tile_pool(name="w3str", bufs=2))
                ps_f1 = ph9.enter_context(
                    tc.tile_pool(name="ps_f1", bufs=4, space="PSUM"))
                with nc.named_scope("p9_ff1"):
                    for jj in range(16):
                        wt = w3str.tile([128, KC, 256], BF16, name="w1t",
                                        tag="w3")
                        f0 = jj * 256
                        eng = nc.scalar if jj % 2 == 0 else nc.sync
                        eng.dma_start(out=wt, in_=f1T_r[:, :, f0:f0 + 256])
                        for fi in range(2):
                            dft = jj * 2 + fi
                            ps = ps_f1.tile([128, 512], F32, name="ps_f",
                                            tag="ps_f")
                            for kc in range(KC):
                                nc.tensor.matmul(
                                    ps, wt[:, kc, fi * 128:fi * 128 + 128],
                                    xlnT[:, kc, :],
                                    start=(kc == 0), stop=(kc == KC - 1))
                            if jj % 2 == 0:
                                nc.scalar.activation(
                                    out=h1T[:, dft, :], in_=ps,
                                    func=AF.Relu,
                                    bias=b1_c[:, dft:dft + 1], scale=1.0)
                            else:
                                nc.vector.tensor_scalar(
                                    out=h1T[:, dft, :], in0=ps,
                                    scalar1=b1_c[:, dft:dft + 1],
                                    scalar2=0.0, op0=OP.add, op1=OP.max)
            xln_stack.close()

            # ============ ff2 + residual + LN2 + out ============
            ff_s = ds_pool.tile([128, NQT, D], F32, name="ff_s")
            with contextlib.ExitStack() as ph10:
                w4str = ph10.enter_context(tc.tile_pool(name="w4str", bufs=8))
                ps_f2 = ph10.enter_context(
                    tc.tile_pool(name="ps_f2", bufs=8, space="PSUM"))
                with nc.named_scope("p10_ff2"):
                    pss = [ps_f2.tile([128, 512], F32, name="ps_g",
                                      tag="ps_g") for _ in range(8)]
                    for nh in range(2):
                        for kc in range(FC):
                            f2h = w4str.tile([128, 512], BF16, name="f2h",
                                             tag="w4")
                            eng = nc.scalar if kc % 2 == 0 else nc.sync
                            eng.dma_start(
                                out=f2h,
                                in_=f2T_r[:, kc, nh * 512:nh * 512 + 512])
                            for qt in range(NQT):
                                nc.tensor.matmul(
                                    pss[2 * qt + nh],
                                    h1T[:, kc, qt * 128:qt * 128 + 128],
                                    f2h, start=(kc == 0), stop=(kc == FC - 1))
                        # fold the xg residual into PSUM on the PE and take
                        # this half's stats while the other half streams;
                        # on the second pass finish LN2 per qt immediately
                        for qt in range(NQT):
                            nc.tensor.matmul(
                                pss[2 * qt + nh], ident_b,
                                xg[:, qt, nh * 512:nh * 512 + 512],
                                start=False, stop=True)
                            nc.vector.bn_stats(
                                out=stats[:, qt, nh, :],
                                in_=pss[2 * qt + nh])
                            if nh == 0:
                                continue
                            nc.vector.bn_aggr(out=mv2[:, qt, :],
                                              in_=stats[:, qt])
                            nc.scalar.activation(out=sd[:, qt:qt + 1],
                                                 in_=mv2[:, qt, 1:2],
                                                 func=AF.Sqrt, bias=eps_t,
                                                 scale=1.0)
                            nc.vector.reciprocal(rstd[:, qt:qt + 1],
                                                 sd[:, qt:qt + 1])
                            x2 = ff_s[:, qt, :]
                            for half in range(2):
                                nc.vector.tensor_scalar(
                                    out=x2[:, half * 512:half * 512 + 512],
                                    in0=pss[2 * qt + half],
                                    scalar1=mv2[:, qt, 0:1],
                                    scalar2=rstd[:, qt:qt + 1],
                                    op0=OP.subtract, op1=OP.mult)
                            ot = ds_pool.tile([128, D], F32, name="out_t",
                                              tag="out_t", bufs=2)
                            nc.vector.tensor_mul(ot[:, 0:512], x2[:, 0:512],
                                                 g2_bc[:, 0:512])
                            nc.vector.tensor_add(ot[:, 0:512], ot[:, 0:512],
                                                 be2_bc[:, 0:512])
                            nc.gpsimd.tensor_mul(ot[:, 512:1024],
                                                 x2[:, 512:1024],
                                                 g2_bc[:, 512:1024])
                            nc.gpsimd.tensor_add(ot[:, 512:1024],
                                                 ot[:, 512:1024],
                                                 be2_bc[:, 512:1024])
                            nc.sync.dma_start(
                                out=out.ap()[qt * 128:qt * 128 + 128, :],
                                in_=ot)
            h1_stack.close()
            ds_stack.close()
            ctx_stack.close()
            scr_stack.close()
            psp_stack.close()

    nc.compile()
    return nc


def _prep_inputs(src, in_proj_w, in_proj_b, out_proj_w, out_proj_b,
                 Qp_w, Qp_b, Kp_w, Kp_b, Vp_w, Vp_b, lam,
                 ff1_w, ff1_b, ff2_w, ff2_b, ln1_g, ln1_b, ln2_g, ln2_b):
    import ml_dtypes
    f = np.float32
    A = lambda x: np.ascontiguousarray(x, dtype=f)
    AB = lambda x: np.ascontiguousarray(np.asarray(x, dtype=f),
                                        dtype=ml_dtypes.bfloat16)
    bias_pack = np.zeros((128, 84), np.float32)
    bias_pack[:, 0:24] = np.asarray(in_proj_b, np.float32).reshape(24, 128).T
    bias_pack[:, 24:32] = np.asarray(Vp_b, np.float32).reshape(8, 128).T
    # ln1 affine folded into ff1: relu((x*g1+be1) @ W1.T + b1)
    #   = relu(x @ (W1*g1).T + (b1 + W1@be1))
    w1f = np.asarray(ff1_w, np.float32)
    g1f = np.asarray(ln1_g, np.float32)
    b1_eff = (np.asarray(ff1_b, np.float32)
              + w1f @ np.asarray(ln1_b, np.float32))
    bias_pack[:, 32:64] = b1_eff.reshape(32, 128).T
    bias_pack[:, 64:72] = np.asarray(ln1_g, np.float32).reshape(8, 128).T
    bias_pack[:, 72:80] = np.asarray(ln1_b, np.float32).reshape(8, 128).T
    bias_pack[0:64, 80] = np.asarray(Qp_b, np.float32)
    bias_pack[0:64, 81] = np.asarray(Kp_b, np.float32)
    sigf = 1.0 / (1.0 + np.exp(-np.float32(np.asarray(lam))))
    bias_pack[:, 82] = sigf
    bias_pack[:, 83] = 1.0 - sigf
    # residual bias row: v-projection biases commute through the
    # (normalized) attention rows, so fold them host-side:
    #   sig*(out_proj_b + v_b @ Wo.T) + (1-sig)*Vp_b
    v_b = np.asarray(in_proj_b, np.float32)[2 * D:3 * D]
    bo_row = (sigf * (np.asarray(out_proj_b, np.float32)
                      + v_b @ np.asarray(out_proj_w, np.float32).T)
              + (1.0 - sigf) * np.asarray(Vp_b, np.float32))
    rows = np.stack([
        bo_row,
        np.asarray(ff2_b, np.float32) + np.asarray(ln1_b, np.float32),
        np.asarray(ln1_g, np.float32),
        np.asarray(ln2_g, np.float32),
        np.asarray(ln2_b, np.float32),
    ]).reshape(-1)
    rows_rep = np.ascontiguousarray(
        np.broadcast_to(rows[None, :], (128, 5 * D)), np.float32)
    shared = {
        "wqkvT": AB(np.asarray(in_proj_w).T),
        "woT": AB(np.asarray(out_proj_w).T),
        "vpT": AB(np.asarray(Vp_w).T),
        "qkpT": AB(np.concatenate([np.asarray(Qp_w).T, np.asarray(Kp_w).T],
                                  axis=1)),
        "f1T": AB((w1f * g1f[None, :]).T),
        "f2T": AB(np.asarray(ff2_w).T),
        "bias_pack": A(bias_pack),
        "rows_pack": rows_rep,
    }
    in_maps = []
    for core in range(8):
        b, h = core // 2, core % 2
        srcb = np.asarray(src[b])
        xTb = srcb.T
        if h == 1:
            # own-query columns first (key order is irrelevant to attention)
            xTb = np.concatenate([xTb[:, SQ:], xTb[:, :SQ]], axis=1)
        m = dict(shared)
        m["xT"] = AB(xTb)
        m["x_own"] = A(srcb[h * SQ:(h + 1) * SQ, :] + bo_row[None, :])
        in_maps.append(m)
    return in_maps


def _run(inputs, trace=False):
    if "nc" not in _cached:
        _cached["nc"] = _build()
    nc = _cached["nc"]
    in_maps = _prep_inputs(**inputs)
    res = run_bass_kernel_spmd(nc, in_maps, core_ids=list(range(8)),
                               trace=trace)
    out = np.empty((B, S, D), np.float32)
    for core in range(8):
        b, h = core // 2, core % 2
        out[b, h * SQ:(h + 1) * SQ, :] = res.results[core]["out"]
    return out, res


def kernel(**inputs) -> np.ndarray:
    out, _ = _run(inputs, trace=False)
    return out



# revision 41
# speedup vs baseline: 1.0573x; 1.0573x over previous
"""Trainium2 Bass kernel for the EnhancedEncoderLayer (dense MHA + low-rank
top-k sparse attention + FFN, two layernorms).

Sharding: 8 cores = (batch b in 0..3) x (query-half h in {0,1}). Each core
computes output rows [b, h*512:(h+1)*512, :]. K/V-side projections are
computed redundantly per batch pair (no cross-core communication).

The host permutes src[b].T columns so each core's own query tokens are
columns 0..511 (attention contracts over all keys, so key order is
irrelevant); this keeps the SPMD program identical across cores.

Precision: all projections run bf16 x bf16 (PSUM accumulation is fp32);
the low-rank sparse-score matmul stays f32r. psp/exp values, the spmm and
the LN2 residual are bf16. Measured rel err ~4.3e-3 vs the 2e-2 gate.

Host-side folds (exact math): projection biases of the v/vsp paths commute
through the (normalized) attention rows into one residual bias row;
sigmoid(lam) is precomputed into the column-bias pack; ln1's affine is
folded into the ff1 weights; all row-broadcast constants are shipped
pre-replicated (no on-device partition broadcasts on the critical path).

Schedule highlights:
- per-head-pair score matmuls target PE row-groups 0:64/64:128 and overlap;
  both land in one 2-bank PSUM tile so a single N=1024 exp evacuates them;
- v/vsp projections computed token-major (stationary = xT tile) - no PE
  transposes; k/q proj PSUM evacuation on DVE, off the exp-saturated ACT;
- the 10-iter top-k threshold bisection (DVE) overlaps dense-path PE work;
- out_proj + spmm + fuse + LN1 run as a per-query-tile pipeline; the
  LN1 output transpose for ff1 uses XBAR DMA-transposes (no PE/DVE cost);
- ff2 streams in two column-half passes; the xg residual is folded into
  PSUM via an identity matmul and LN2 finishes per qt inside the second
  pass, overlapping the tail.
"""
import sys
import os
import contextlib

for _p in ('/opt/trn_rl_repo',):
    if _p not in sys.path:
        sys.path.insert(0, _p)

import numpy as np
import concourse.bacc as bacc
import concourse.tile as tile
from concourse import mybir
from concourse.bass_utils import run_bass_kernel_spmd
from concourse.masks import make_identity

F32 = mybir.dt.float32
F32R = mybir.dt.float32r
BF16 = mybir.dt.bfloat16
AF = mybir.ActivationFunctionType
OP = mybir.AluOpType

B, S, D, H, R, DFF = 4, 1024, 1024, 16, 64, 4096
DH = D // H          # 64
SQ = S // 2          # 512 own queries per core
KK = max(1, int(S * 0.2))   # 204
KC = D // 128        # 8 contraction chunks over D
FC = DFF // 128      # 32 chunks over DFF
NQT = SQ // 128      # 4 query tiles
NTOK = S // 128      # 8 token tiles
BISECT_ITERS = 10
INV_SQRT = 0.125     # 1/sqrt(DH) == 1/sqrt(R)

_cached = {}


def _build():
    nc = bacc.Bacc()

    def din(name, shape):
        return nc.declare_dram_parameter(name, list(shape), F32, isOutput=False)

    xT = nc.declare_dram_parameter("xT", [D, S], BF16, isOutput=False)
    x_own = din("x_own", [SQ, D])   # own rows, token-major (f32 residual)
    wqkvT = nc.declare_dram_parameter("wqkvT", [D, 3 * D], BF16,
                                      isOutput=False)
    woT = nc.declare_dram_parameter("woT", [D, D], BF16, isOutput=False)
    vpT = nc.declare_dram_parameter("vpT", [D, D], BF16, isOutput=False)
    qkpT = nc.declare_dram_parameter("qkpT", [D, 2 * R], BF16,
                                     isOutput=False)
    f1T = nc.declare_dram_parameter("f1T", [D, DFF], BF16, isOutput=False)
    f2T = nc.declare_dram_parameter("f2T", [DFF, D], BF16, isOutput=False)
    # column-bias pack (host-side layout, one contiguous DMA):
    # [:, 0:24]=in_proj_b  [:, 24:32]=Vp_b  [:, 32:64]=ff1_b
    # [:, 64:72]=ln1_g  [:, 72:80]=ln1_b  [0:64, 80]=Qp_b  [0:64, 81]=Kp_b
    bias_pack = din("bias_pack", [128, 84])
    # host-replicated row constants: 0=residual bias row, 1=b2+ln1_b,
    # 2=ln1_g, 3=ln2_g, 4=ln2_b
    rows_pack = din("rows_pack", [128, 5 * D])
    out = nc.declare_dram_parameter("out", [SQ, D], F32, isOutput=True)
    DBG = bool(os.environ.get("BASSK_DEBUG"))
    if DBG:
        dbg_dense = nc.declare_dram_parameter("dbg_dense", [SQ, D], F32,
                                              isOutput=True)
        dbg_sparse = nc.declare_dram_parameter("dbg_sparse", [SQ, D], F32,
                                               isOutput=True)
        dbg_lo = nc.declare_dram_parameter("dbg_lo", [128, NQT], F32,
                                           isOutput=True)
        dbg_rs = nc.declare_dram_parameter("dbg_rs", [128, NQT], F32,
                                           isOutput=True)

    xT_r = xT.ap().rearrange("(kc p) s -> p kc s", p=128)
    wqkvT_r = wqkvT.ap().rearrange("(kc p) f -> p kc f", p=128)
    woT_r = woT.ap().rearrange("(kc p) f -> p kc f", p=128)
    vpT_r = vpT.ap().rearrange("(kc p) f -> p kc f", p=128)
    qkpT_r = qkpT.ap().rearrange("(kc p) f -> p kc f", p=128)
    f1T_r = f1T.ap().rearrange("(kc p) f -> p kc f", p=128)
    f2T_r = f2T.ap().rearrange("(kc p) f -> p kc f", p=128)

    with tile.TileContext(nc) as tc:
        est = contextlib.ExitStack()
        with est:
            # ---------------- constants ----------------
            consts = est.enter_context(tc.tile_pool(name="consts", bufs=1))

            ident_f = consts.tile([128, 128], F32, name="ident_f")
            make_identity(nc, ident_f)
            ident_b = consts.tile([128, 128], BF16, name="ident_b")
            nc.vector.tensor_copy(out=ident_b, in_=ident_f)

            eps_t = consts.tile([128, 1], F32, name="eps_t")
            nc.vector.memset(eps_t, 1e-5)
            ones1 = consts.tile([128, 1], F32, name="ones1")
            nc.vector.memset(ones1, 1.0)
            ones16 = consts.tile([128, 16], F32, name="ones16")
            nc.vector.memset(ones16, 1.0)

            bp = consts.tile([128, 84], F32, name="bias_pack")
            bqkv_c = bp[:, 0:24]
            bvp_c = bp[:, 24:32]
            b1_c = bp[:, 32:64]
            g1_c = bp[:, 64:72]
            be1_c = bp[:, 72:80]
            bqp_c = bp[0:64, 80:81]
            bkp_c = bp[0:64, 81:82]
            sig_bc = bp[:, 82:83]
            oms_bc = bp[:, 83:84]
            ones1b = consts.tile([128, 1], BF16, name="ones1b")
            nc.vector.memset(ones1b, 1.0)
            sd_pre = consts.tile([1, 1], F32, name="sd_pre")

            def load_bias_cols():
                # one contiguous transfer on the (otherwise idle) Pool ring
                nc.gpsimd.dma_start(out=bp, in_=bias_pack.ap())


            # own-token residual (+ sig*bo); loaded after xT is in flight
            xot_pool = est.enter_context(tc.tile_pool(name="xot_pool",
                                                      bufs=1))
            xot = xot_pool.tile([128, NQT, D], F32, name="xot")

            bis = est.enter_context(tc.tile_pool(name="bis", bufs=1))
            lo = bis.tile([128, NQT], F32, name="lo")
            hi = bis.tile([128, NQT], F32, name="hi")
            mid = bis.tile([128, NQT], F32, name="mid")
            cnts = bis.tile([128, NQT], F32, name="cnts")
            pred = bis.tile([128, NQT], mybir.dt.uint32, name="pred")
            rs_sp = bis.tile([128, NQT], F32, name="rs_sp")
            rcp_sp = bis.tile([128, NQT], F32, name="rcp_sp")

            # long-lived activation groups (left stack)
            sp_stack = contextlib.ExitStack()
            sp_pool = sp_stack.enter_context(
                tc.tile_pool(name="sp_pool", bufs=1))
            Vsp = sp_pool.tile([128, NTOK, D], BF16, name="Vsp")
            kspT = sp_pool.tile([64, S], F32R, name="kspT")
            qspT = sp_pool.tile([64, SQ], F32R, name="qspT")

            wo_full = sp_pool.tile([128, KC, D], BF16, name="wo_full")

            dn_stack = contextlib.ExitStack()
            dn_pool = dn_stack.enter_context(
                tc.tile_pool(name="dn_pool", bufs=1))
            kT = dn_pool.tile([128, KC, S], BF16, name="kT")
            Vaug = dn_pool.tile([128, NTOK, H * (DH + 1)], BF16, name="Vaug")
            qT = dn_pool.tile([128, KC, SQ], BF16, name="qT")

            Vaug_h = Vaug.rearrange("p t (h c) -> p t h c", c=DH + 1)
            for t in range(NTOK):
                nc.vector.tensor_copy(out=Vaug_h[:, t, :, DH:DH + 1],
                                      in_=ones16)

            # right-stack pools (all close together after phase 7)
            psp_stack = contextlib.ExitStack()
            psp_pool = psp_stack.enter_context(
                tc.tile_pool(name="psp_pool", bufs=1, side="right"))
            psp = [psp_pool.tile([128, S], BF16, name=f"psp{qt}")
                   for qt in range(NQT)]
            scr_stack = contextlib.ExitStack()
            scr_pool = scr_stack.enter_context(
                tc.tile_pool(name="scr", bufs=1, side="right"))
            ctx_stack = contextlib.ExitStack()
            ctx_pool = ctx_stack.enter_context(
                tc.tile_pool(name="ctx_pool", bufs=1, side="right"))
            ctxT = ctx_pool.tile([128, KC, SQ], BF16, name="ctxT")
            # ============ projections + sparse path + attention ============
            with contextlib.ExitStack() as ph0:
                xt_pool = ph0.enter_context(
                    tc.tile_pool(name="xt_pool", bufs=1))
                wstr = ph0.enter_context(tc.tile_pool(name="wstr", bufs=8))
                pt_pool = ph0.enter_context(
                    tc.tile_pool(name="pt_pool", bufs=4))
                rc_pool = ph0.enter_context(
                    tc.tile_pool(name="rc_pool", bufs=1))
                ps_a = ph0.enter_context(
                    tc.tile_pool(name="ps_a", bufs=2, space="PSUM"))
                ps_b = ph0.enter_context(
                    tc.tile_pool(name="ps_b", bufs=4, space="PSUM"))

                # small sparse weights first, then xT on both queues
                qkpt = wstr.tile([128, KC, 2 * R], BF16, name="qkpt",
                                 tag="wsmall")
                nc.sync.dma_start(out=qkpt, in_=qkpT_r)
                qpt = qkpt[:, :, 0:R]
                kpt = qkpt[:, :, R:2 * R]
                xTt = xt_pool.tile([128, KC, S], BF16, name="xTt")
                for kc2 in range(4):
                    eng = nc.scalar if kc2 % 2 == 0 else nc.sync
                    eng.dma_start(out=xTt[:, 2 * kc2:2 * kc2 + 2, :],
                                  in_=xT_r[:, 2 * kc2:2 * kc2 + 2, :])
                load_bias_cols()
                vw_stack = contextlib.ExitStack()
                vw_pool = vw_stack.enter_context(
                    tc.tile_pool(name="vw_pool", bufs=2))

                # ---- sparse projections + scores ----
                with nc.named_scope("p0_ksp_qsp"):
                    ka = ps_a.tile([128, 1024], F32, name="ksa", tag="psa")
                    for nh in range(2):
                        for kc in range(KC):
                            nc.tensor.matmul(
                                ka[0:64, nh * 512:nh * 512 + 512],
                                kpt[:, kc, :],
                                xTt[:, kc, nh * 512:nh * 512 + 512],
                                start=(kc == 0), stop=(kc == KC - 1))
                    nc.scalar.activation(
                        out=kspT, in_=ka[0:64, :], func=AF.Identity,
                        bias=bkp_c, scale=1.0)
                    ps = ps_b.tile([128, 512], F32, name="ps", tag="psb")
                    for kc in range(KC):
                        nc.tensor.matmul(ps[0:64, :], qpt[:, kc, :],
                                         xTt[:, kc, 0:SQ],
                                         start=(kc == 0), stop=(kc == KC - 1))
                    nc.scalar.activation(out=qspT, in_=ps[0:64, :],
                                         func=AF.Identity, bias=bqp_c,
                                         scale=1.0)

                with nc.named_scope("p2_ssp"):
                    for qt in range(NQT):
                        ps2 = ps_a.tile([128, 1024], F32, name="ps2",
                                        tag="psa")
                        for nh in range(2):
                            nc.tensor.matmul(
                                ps2[:, nh * 512:nh * 512 + 512],
                                qspT[:, qt * 128:qt * 128 + 128],
                                kspT[:, nh * 512:nh * 512 + 512],
                                start=True, stop=True)
                        nc.scalar.activation(
                            out=psp[qt], in_=ps2, func=AF.Exp,
                            scale=INV_SQRT)

                # ---- top-k threshold bisection (DVE; overlaps PE below) ----
                with nc.named_scope("p3_bisect"):
                    nc.vector.memset(lo, 0.0)
                    nc.vector.memset(hi, 16.0)
                    for it in range(BISECT_ITERS):
                        nc.vector.tensor_add(mid, lo, hi)
                        nc.vector.tensor_scalar_mul(mid, mid, 0.5)
                        for qt in range(NQT):
                            scr = scr_pool.tile([128, S], BF16, name="scr",
                                                tag="scr")
                            nc.vector.scalar_tensor_tensor(
                                out=scr, in0=psp[qt],
                                scalar=mid[:, qt:qt + 1],
                                in1=ones1b.to_broadcast([128, S]),
                                op0=OP.is_ge, op1=OP.mult,
                                accum_out=cnts[:, qt:qt + 1])
                        nc.vector.tensor_scalar(out=pred, in0=cnts,
                                                scalar1=float(KK),
                                                scalar2=None, op0=OP.is_ge)
                        nc.vector.copy_predicated(lo, pred, mid)
                        nc.vector.tensor_scalar(out=pred, in0=cnts,
                                                scalar1=float(KK),
                                                scalar2=None, op0=OP.is_lt)
                        nc.vector.copy_predicated(hi, pred, mid)
                    for qt in range(NQT):
                        nc.vector.scalar_tensor_tensor(
                            out=psp[qt], in0=psp[qt],
                            scalar=lo[:, qt:qt + 1],
                            in1=psp[qt], op0=OP.is_ge, op1=OP.mult,
                            accum_out=rs_sp[:, qt:qt + 1])
                    if DBG:
                        nc.sync.dma_start(out=dbg_lo.ap(), in_=lo)
                        nc.sync.dma_start(out=dbg_rs.ap(), in_=rs_sp)
                    nc.vector.tensor_scalar(out=rs_sp, in0=rs_sp,
                                            scalar1=1e-9, scalar2=None,
                                            op0=OP.add)
                    nc.vector.reciprocal(rcp_sp, rs_sp)
                    nc.vector.tensor_scalar_mul(rcp_sp, rcp_sp, oms_bc)

                for qt in range(NQT):
                    nc.gpsimd.dma_start(
                        out=xot[:, qt, :],
                        in_=x_own.ap()[qt * 128:qt * 128 + 128, :])

                _wc_cnt = [0]

                def w_chunk(w_view, f0, nfs=128):
                    wt = wstr.tile([128, KC, 128], BF16, name="wt", tag="wt")
                    eng = nc.sync if _wc_cnt[0] % 2 == 0 else nc.scalar
                    _wc_cnt[0] += 1
                    eng.dma_start(out=wt[:, :, :nfs],
                                  in_=w_view[:, :, f0:f0 + nfs])
                    return wt

                # ---- v / vsp projections, directly token-major ----
                # stationary = xT token-tile chunk, moving = weight rows, so
                # the PSUM result lands token-major (no transposes, no
                # copies). Projection biases are folded into the residual
                # row host-side (softmax rows sum to 1).
                def proj_tokmajor(w_view, f_lo, to_vaug, scope):
                    with nc.named_scope(scope):
                        for fh in range(2):
                            wh = vw_pool.tile([128, KC, 512], BF16,
                                              name="wh", tag="wh")
                            eng = nc.scalar if fh == 0 else nc.sync
                            eng.dma_start(
                                out=wh,
                                in_=w_view[:, :,
                                           f_lo + fh * 512:
                                           f_lo + fh * 512 + 512])
                            for t in range(NTOK):
                                ps = ps_b.tile([128, 512], F32, name="psv",
                                               tag="psb")
                                for kc in range(KC):
                                    nc.tensor.matmul(
                                        ps,
                                        xTt[:, kc, t * 128:t * 128 + 128],
                                        wh[:, kc, :],
                                        start=(kc == 0), stop=(kc == KC - 1))
                                if to_vaug:
                                    nc.scalar.activation(
                                        out=Vaug_h[:, t, 8 * fh:8 * fh + 8,
                                                   0:DH],
                                        in_=ps, func=AF.Identity, scale=1.0)
                                else:
                                    nc.scalar.activation(
                                        out=Vsp[:, t,
                                                fh * 512:fh * 512 + 512],
                                        in_=ps, func=AF.Identity, scale=1.0)

                proj_tokmajor(wqkvT_r, 2 * D, True, "p0_v")
                proj_tokmajor(vpT_r, 0, False, "p0_vsp")
                vw_stack.close()

                # out_proj weights (needed only after attention)
                nc.scalar.dma_start(out=wo_full, in_=woT_r)

                # ---- interleaved k/q projections + dense attention ----
                with nc.named_scope("p4_kq_attn"):
                    for jj in range(4):
                        for fi in range(2):
                            ft = jj * 2 + fi
                            wkc = w_chunk(wqkvT_r, D + ft * 128)
                            for nh in range(2):
                                ps = ps_b.tile([128, 512], F32, name="ps",
                                               tag="psb")
                                for kc in range(KC):
                                    nc.tensor.matmul(
                                        ps,
                                        wkc[:, kc, 0:128],
                                        xTt[:, kc, nh * 512:nh * 512 + 512],
                                        start=(kc == 0), stop=(kc == KC - 1))
                                nc.vector.tensor_scalar(
                                    out=kT[:, ft, nh * 512:nh * 512 + 512],
                                    in0=ps,
                                    scalar1=bqkv_c[:, 8 + ft:8 + ft + 1],
                                    scalar2=None, op0=OP.add)
                            wqc = w_chunk(wqkvT_r, ft * 128)
                            ps = ps_b.tile([128, 512], F32, name="ps",
                                           tag="psb")
                            for kc in range(KC):
                                nc.tensor.matmul(
                                    ps, wqc[:, kc, 0:128],
                                    xTt[:, kc, 0:SQ],
                                    start=(kc == 0), stop=(kc == KC - 1))
                            nc.vector.tensor_scalar(
                                out=qT[:, ft, :], in0=ps,
                                scalar1=bqkv_c[:, ft:ft + 1],
                                scalar2=None, op0=OP.add)
                        # attention for the 4 heads of these two f-tiles.
                        # The two heads of an f-tile sit on partition rows
                        # 0:64 / 64:128, so their score matmuls target
                        # different PE row-groups and overlap when issued
                        # back-to-back; both land in one 2-bank PSUM tile so
                        # a single N=1024 exp evacuates the pair.
                        for ft in (2 * jj, 2 * jj + 1):
                            pctx = {po: ps_b.tile([128, 512], F32,
                                                  name="ps_c", tag="psb")
                                    for po in (0, 64)}
                            for t in range(NTOK):
                                ps2 = ps_a.tile([128, 1024], F32,
                                                name="ps_s", tag="psa")
                                for po in (0, 64):
                                    nc.tensor.matmul(
                                        ps2[:, 8 * po:8 * po + 512],
                                        kT[po:po + 64, ft,
                                           t * 128:t * 128 + 128],
                                        qT[po:po + 64, ft, :],
                                        start=True, stop=True)
                                pt = pt_pool.tile([128, 1024], BF16,
                                                  name="pT", tag="pT")
                                nc.scalar.activation(out=pt, in_=ps2,
                                                     func=AF.Exp,
                                                     scale=INV_SQRT)
                                for po in (0, 64):
                                    hh = 2 * ft + po // 64
                                    nc.tensor.matmul(
                                        pctx[po][0:65, :],
                                        Vaug[:, t, hh * 65:hh * 65 + 65],
                                        pt[:, 8 * po:8 * po + 512],
                                        start=(t == 0),
                                        stop=(t == NTOK - 1))
                            for po in (0, 64):
                                rsr = rc_pool.tile([1, 512], F32, name="rsr",
                                                   tag="rsr")
                                nc.vector.tensor_copy(out=rsr,
                                                      in_=pctx[po][64:65, :])
                                rch = rc_pool.tile([1, 512], F32, name="rch",
                                                   tag="rch")
                                nc.vector.reciprocal_approx_fast(out=rch,
                                                                 in_=rsr)
                                rb = rc_pool.tile([64, 512], F32, name="rb",
                                                  tag="rb")
                                nc.gpsimd.partition_broadcast(rb, rch)
                                nc.vector.tensor_mul(
                                    out=ctxT[po:po + 64, ft, :],
                                    in0=pctx[po][0:64, :], in1=rb)

                # pull the exp->sqrt ACT table switch off the LN1 chain
                nc.scalar.activation(out=sd_pre, in_=eps_t[0:1, :],
                                     func=AF.Sqrt, bias=eps_t[0:1, :],
                                     scale=1.0)

            dn_stack.close()   # free kT, Vaug, qT

            ds_stack = contextlib.ExitStack()
            ds_pool = ds_stack.enter_context(
                tc.tile_pool(name="ds_pool", bufs=1, side="right"))
            dense_s = ds_pool.tile([128, NQT, D], F32, name="dense_s")
            sparse_s = ds_pool.tile([128, NQT, D], F32, name="sparse_s")

            # ---- host-replicated row constants (one DMA on the idle
            # sync ring; lands mid-attention) ----
            rows_t = ds_pool.tile([128, 5, D], F32, name="rows_t")
            nc.sync.dma_start(
                out=rows_t,
                in_=rows_pack.ap().rearrange("p (r d) -> p r d", r=5))
            b12_bc = rows_t[:, 1, :]
            g1_bc = rows_t[:, 2, :]
            g2_bc = rows_t[:, 3, :]
            be2_bc = rows_t[:, 4, :]

            xg = ds_pool.tile([128, NQT, D], BF16, name="xg")
            stats = ds_pool.tile([128, NQT, 2, 6], F32, name="stats")
            mv2 = ds_pool.tile([128, NQT, 2], F32, name="mv2")
            sd = ds_pool.tile([128, NQT], F32, name="sd")
            rstd = ds_pool.tile([128, NQT], F32, name="rstd")
            x1s = ds_pool.tile([128, 2, D], F32, name="x1s")
            xhat_bf = ds_pool.tile([128, NQT, D], BF16, name="xhat_bf")

            def ln_normalize(x1, qt, out=None):
                for half in range(2):
                    nc.vector.bn_stats(
                        out=stats[:, qt, half, :],
                        in_=x1[:, half * 512:half * 512 + 512])
                nc.vector.bn_aggr(out=mv2[:, qt, :], in_=stats[:, qt])
                nc.scalar.activation(out=sd[:, qt:qt + 1],
                                     in_=mv2[:, qt, 1:2], func=AF.Sqrt,
                                     bias=eps_t, scale=1.0)
                nc.vector.reciprocal(rstd[:, qt:qt + 1], sd[:, qt:qt + 1])
                nc.vector.tensor_scalar(out=x1 if out is None else out,
                                        in0=x1,
                                        scalar1=mv2[:, qt, 0:1],
                                        scalar2=rstd[:, qt:qt + 1],
                                        op0=OP.subtract, op1=OP.mult)

            xln_stack = contextlib.ExitStack()
            xln_pool = xln_stack.enter_context(
                tc.tile_pool(name="xln_pool", bufs=1, side="right"))
            xlnT = xln_pool.tile([128, KC, SQ], BF16, name="xlnT")

            # ====== per-qt pipeline: out_proj + spmm + fuse + LN1 + xT ======
            with contextlib.ExitStack() as ph5:
                pm_pool = ph5.enter_context(
                    tc.tile_pool(name="pm_pool", bufs=1))
                ps_tr2 = ph5.enter_context(
                    tc.tile_pool(name="ps_tr2", bufs=4, space="PSUM"))
                ps_mm = ph5.enter_context(
                    tc.tile_pool(name="ps_mm", bufs=4, space="PSUM"))
                pmT = pm_pool.tile([128, NTOK, SQ], BF16, name="pmT")
                with nc.named_scope("p5_outproj"):
                    # all masked-p transposes first: they depend only on
                    # psp, so the PE stays busy while the attention tail's
                    # DVE normalize chain drains
                    for qt in range(NQT):
                        for t in range(NTOK):
                            pst = ps_tr2.tile([128, 128], BF16, name="pst2",
                                              tag="pst2")
                            nc.tensor.transpose(
                                pst, psp[qt][:, t * 128:t * 128 + 128],
                                ident_b)
                            nc.scalar.copy(
                                out=pmT[:, t, qt * 128:qt * 128 + 128],
                                in_=pst)
                    for qt in range(NQT):
                        for nh in range(2):
                            ps = ps_mm.tile([128, 512], F32, name="ps_o",
                                            tag="ps_o")
                            for t in range(NTOK):
                                nc.tensor.matmul(
                                    ps,
                                    pmT[:, t, qt * 128:qt * 128 + 128],
                                    Vsp[:, t, nh * 512:nh * 512 + 512],
                                    start=(t == 0), stop=(t == NTOK - 1))
                            nc.scalar.activation(
                                out=sparse_s[:, qt, nh * 512:nh * 512 + 512],
                                in_=ps, func=AF.Copy,
                                scale=rcp_sp[:, qt:qt + 1])
                        # dense out_proj for this query tile
                        pss2 = [ps_mm.tile([128, 512], F32, name="ps_o",
                                           tag="ps_o") for _ in range(2)]
                        for kc in range(KC):
                            for nh in range(2):
                                nc.tensor.matmul(
                                    pss2[nh],
                                    ctxT[:, kc, qt * 128:qt * 128 + 128],
                                    wo_full[:, kc, nh * 512:nh * 512 + 512],
                                    start=(kc == 0), stop=(kc == KC - 1))
                        for nh in range(2):
                            nc.scalar.activation(
                                out=dense_s[:, qt, nh * 512:nh * 512 + 512],
                                in_=pss2[nh], func=AF.Copy, scale=sig_bc)
                        # fuse + LN1 + transpose for ff1
                        x1 = x1s[:, qt % 2, :]
                        nc.vector.tensor_add(x1[:, 0:512],
                                             dense_s[:, qt, 0:512],
                                             sparse_s[:, qt, 0:512])
                        nc.gpsimd.tensor_add(x1[:, 512:1024],
                                             dense_s[:, qt, 512:1024],
                                             sparse_s[:, qt, 512:1024])
                        nc.vector.tensor_add(x1[:, 0:512], x1[:, 0:512],
                                             xot[:, qt, 0:512])
                        nc.gpsimd.tensor_add(x1[:, 512:1024],
                                             x1[:, 512:1024],
                                             xot[:, qt, 512:1024])
                        ln_normalize(x1, qt, out=xhat_bf[:, qt, :])
                        nc.sync.dma_start_transpose(
                            out=xlnT[:, :, qt * 128:qt * 128 + 128],
                            in_=xhat_bf[:, qt, :])
            if DBG:
                for qt in range(NQT):
                    nc.sync.dma_start(
                        out=dbg_dense.ap()[qt * 128:qt * 128 + 128, :],
                        in_=dense_s[:, qt, :])
                    nc.sync.dma_start(
                        out=dbg_sparse.ap()[qt * 128:qt * 128 + 128, :],
                        in_=sparse_s[:, qt, :])
            sp_stack.close()

            # xg (LN2 residual) on DVE while ff1 owns the PE
            for qt in range(NQT):
                nc.vector.tensor_mul(xg[:, qt, :], xhat_bf[:, qt, :], g1_bc)
                nc.vector.tensor_add(xg[:, qt, :], xg[:, qt, :], b12_bc)

            # ============ ff1 + relu ============
            h1_stack = contextlib.ExitStack()
            h1_pool = h1_stack.enter_context(
                tc.tile_pool(name="h1_pool", bufs=1))
            h1T = h1_pool.tile([128, FC, SQ], BF16, name="h1T")
            with contextlib.ExitStack() as ph9:
                w3str = ph9.enter_context(tc.tile_pool(name="w3str", bufs=4))
                ps_f1 = ph9.enter_context(
                    tc.tile_pool(name="ps_f1", bufs=4, space="PSUM"))
                with nc.named_scope("p9_ff1"):
                    for jj in range(16):
                        wt = w3str.tile([128, KC, 256], BF16, name="w1t",
                                        tag="w3")
                        f0 = jj * 256
                        eng = nc.scalar if jj % 2 == 0 else nc.sync
                        eng.dma_start(out=wt, in_=f1T_r[:, :, f0:f0 + 256])
                        for fi in range(2):
                            dft = jj * 2 + fi
                            ps = ps_f1.tile([128, 512], F32, name="ps_f",
                                            tag="ps_f")
                            for kc in range(KC):
                                nc.tensor.matmul(
                                    ps, wt[:, kc, fi * 128:fi * 128 + 128],
                                    xlnT[:, kc, :],
                                    start=(kc == 0), stop=(kc == KC - 1))
                            if jj % 2 == 0:
                                nc.scalar.activation(
                                    out=h1T[:, dft, :], in_=ps,
                                    func=AF.Relu,
                                    bias=b1_c[:, dft:dft + 1], scale=1.0)
                            else:
                                nc.vector.tensor_scalar(
                                    out=h1T[:, dft, :], in0=ps,
                                    scalar1=b1_c[:, dft:dft + 1],
                                    scalar2=0.0, op0=OP.add, op1=OP.max)
            xln_stack.close()

            # ============ ff2 + residual + LN2 + out ============
            ff_s = ds_pool.tile([128, NQT, D], F32, name="ff_s")
            with contextlib.ExitStack() as ph10:
                w4str = ph10.enter_context(tc.tile_pool(name="w4str",
                                                        bufs=8))
                w4b_pool = ph10.enter_context(
                    tc.tile_pool(name="w4b_pool", bufs=1))
                ps_f2 = ph10.enter_context(
                    tc.tile_pool(name="ps_f2", bufs=8, space="PSUM"))
                with nc.named_scope("p10_ff2"):
                    pss = [ps_f2.tile([128, 512], F32, name="ps_g",
                                      tag="ps_g") for _ in range(8)]
                    # prefetch the second column-half of f2T (resident for
                    # the qt-major second pass)
                    f2b = w4b_pool.tile([128, FC, 512], BF16, name="f2b")
                    for kc in range(FC):
                        eng = nc.scalar if kc % 2 == 0 else nc.sync
                        eng.dma_start(
                            out=f2b[:, kc, :],
                            in_=f2T_r[:, kc, 512:1024])
                    # pass 0: stream the first half kc-major
                    for kc in range(FC):
                        f2h = w4str.tile([128, 512], BF16, name="f2h",
                                         tag="w4")
                        eng = nc.scalar if kc % 2 == 0 else nc.sync
                        eng.dma_start(out=f2h,
                                      in_=f2T_r[:, kc, 0:512])
                        for qt in range(NQT):
                            nc.tensor.matmul(
                                pss[2 * qt],
                                h1T[:, kc, qt * 128:qt * 128 + 128],
                                f2h, start=(kc == 0), stop=(kc == FC - 1))
                    for qt in range(NQT):
                        nc.tensor.matmul(
                            pss[2 * qt], ident_b, xg[:, qt, 0:512],
                            start=False, stop=True)
                        nc.vector.bn_stats(out=stats[:, qt, 0, :],
                                           in_=pss[2 * qt])
                    # pass 1: qt-major on the resident half; finish LN2 and
                    # store per qt while later qt's matmuls run
                    for qt in range(NQT):
                        for kc in range(FC):
                            nc.tensor.matmul(
                                pss[2 * qt + 1],
                                h1T[:, kc, qt * 128:qt * 128 + 128],
                                f2b[:, kc, :],
                                start=(kc == 0), stop=(kc == FC - 1))
                        nc.tensor.matmul(
                            pss[2 * qt + 1], ident_b, xg[:, qt, 512:1024],
                            start=False, stop=True)
                        nc.vector.bn_stats(out=stats[:, qt, 1, :],
                                           in_=pss[2 * qt + 1])
                        nc.vector.bn_aggr(out=mv2[:, qt, :],
                                          in_=stats[:, qt])
                        nc.scalar.activation(out=sd[:, qt:qt + 1],
                                             in_=mv2[:, qt, 1:2],
                                             func=AF.Sqrt, bias=eps_t,
                                             scale=1.0)
                        nc.vector.reciprocal(rstd[:, qt:qt + 1],
                                             sd[:, qt:qt + 1])
                        x2 = ff_s[:, qt, :]
                        for half in range(2):
                            nc.vector.tensor_scalar(
                                out=x2[:, half * 512:half * 512 + 512],
                                in0=pss[2 * qt + half],
                                scalar1=mv2[:, qt, 0:1],
                                scalar2=rstd[:, qt:qt + 1],
                                op0=OP.subtract, op1=OP.mult)
                        ot = ds_pool.tile([128, D], F32, name="out_t",
                                          tag="out_t", bufs=2)
                        nc.vector.tensor_mul(ot[:, 0:512], x2[:, 0:512],
                                             g2_bc[:, 0:512])
                        nc.vector.tensor_add(ot[:, 0:512], ot[:, 0:512],
                                             be2_bc[:, 0:512])
                        nc.gpsimd.tensor_mul(ot[:, 512:1024],
                                             x2[:, 512:1024],
                                             g2_bc[:, 512:1024])
                        nc.gpsimd.tensor_add(ot[:, 512:1024],
                                             ot[:, 512:1024],
                                             be2_bc[:, 512:1024])
                        nc.sync.dma_start(
                            out=out.ap()[qt * 128:qt * 128 + 128, :],
                            in_=ot)
            h1_stack.close()
            ds_stack.close()
            ctx_stack.close()
            scr_stack.close()
            psp_stack.close()

    nc.compile()
    return nc


def _prep_inputs(src, in_proj_w, in_proj_b, out_proj_w, out_proj_b,
                 Qp_w, Qp_b, Kp_w, Kp_b, Vp_w, Vp_b, lam,
                 ff1_w, ff1_b, ff2_w, ff2_b, ln1_g, ln1_b, ln2_g, ln2_b):
    import ml_dtypes
    f = np.float32
    A = lambda x: np.ascontiguousarray(x, dtype=f)
    AB = lambda x: np.ascontiguousarray(np.asarray(x, dtype=f),
                                        dtype=ml_dtypes.bfloat16)
    bias_pack = np.zeros((128, 84), np.float32)
    bias_pack[:, 0:24] = np.asarray(in_proj_b, np.float32).reshape(24, 128).T
    bias_pack[:, 24:32] = np.asarray(Vp_b, np.float32).reshape(8, 128).T
    # ln1 affine folded into ff1: relu((x*g1+be1) @ W1.T + b1)
    #   = relu(x @ (W1*g1).T + (b1 + W1@be1))
    w1f = np.asarray(ff1_w, np.float32)
    g1f = np.asarray(ln1_g, np.float32)
    b1_eff = (np.asarray(ff1_b, np.float32)
              + w1f @ np.asarray(ln1_b, np.float32))
    bias_pack[:, 32:64] = b1_eff.reshape(32, 128).T
    bias_pack[:, 64:72] = np.asarray(ln1_g, np.float32).reshape(8, 128).T
    bias_pack[:, 72:80] = np.asarray(ln1_b, np.float32).reshape(8, 128).T
    bias_pack[0:64, 80] = np.asarray(Qp_b, np.float32)
    bias_pack[0:64, 81] = np.asarray(Kp_b, np.float32)
    sigf = 1.0 / (1.0 + np.exp(-np.float32(np.asarray(lam))))
    bias_pack[:, 82] = sigf
    bias_pack[:, 83] = 1.0 - sigf
    # residual bias row: v-projection biases commute through the
    # (normalized) attention rows, so fold them host-side:
    #   sig*(out_proj_b + v_b @ Wo.T) + (1-sig)*Vp_b
    v_b = np.asarray(in_proj_b, np.float32)[2 * D:3 * D]
    bo_row = (sigf * (np.asarray(out_proj_b, np.float32)
                      + v_b @ np.asarray(out_proj_w, np.float32).T)
              + (1.0 - sigf) * np.asarray(Vp_b, np.float32))
    rows = np.stack([
        bo_row,
        np.asarray(ff2_b, np.float32) + np.asarray(ln1_b, np.float32),
        np.asarray(ln1_g, np.float32),
        np.asarray(ln2_g, np.float32),
        np.asarray(ln2_b, np.float32),
    ]).reshape(-1)
    rows_rep = np.ascontiguousarray(
        np.broadcast_to(rows[None, :], (128, 5 * D)), np.float32)
    shared = {
        "wqkvT": AB(np.asarray(in_proj_w).T),
        "woT": AB(np.asarray(out_proj_w).T),
        "vpT": AB(np.asarray(Vp_w).T),
        "qkpT": AB(np.concatenate([np.asarray(Qp_w).T, np.asarray(Kp_w).T],
                                  axis=1)),
        "f1T": AB((w1f * g1f[None, :]).T),
        "f2T": AB(np.asarray(ff2_w).T),
        "bias_pack": A(bias_pack),
        "rows_pack": rows_rep,
    }
    in_maps = []
    for core in range(8):
        b, h = core // 2, core % 2
        srcb = np.asarray(src[b])
        xTb = srcb.T
        if h == 1:
            # own-query columns first (key order is irrelevant to attention)
            xTb = np.concatenate([xTb[:, SQ:], xTb[:, :SQ]], axis=1)
        m = dict(shared)
        m["xT"] = AB(xTb)
        m["x_own"] = A(srcb[h * SQ:(h + 1) * SQ, :] + bo_row[None, :])
        in_maps.append(m)
    return in_maps


def _run(inputs, trace=False):
    if "nc" not in _cached:
        _cached["nc"] = _build()
    nc = _cached["nc"]
    in_maps = _prep_inputs(**inputs)
    res = run_bass_kernel_spmd(nc, in_maps, core_ids=list(range(8)),
                               trace=trace)
    out = np.empty((B, S, D), np.float32)
    for core in range(8):
        b, h = core // 2, core % 2
        out[b, h * SQ:(h + 1) * SQ, :] = res.results[core]["out"]
    return out, res


def kernel(**inputs) -> np.ndarray:
    out, _ = _run(inputs, trace=False)
    return out



# revision 42
# speedup vs baseline: 1.0747x; 1.0165x over previous
"""Trainium2 Bass kernel for the EnhancedEncoderLayer (dense MHA + low-rank
top-k sparse attention + FFN, two layernorms).

Sharding: 8 cores = (batch b in 0..3) x (query-half h in {0,1}). Each core
computes output rows [b, h*512:(h+1)*512, :]. K/V-side projections are
computed redundantly per batch pair (no cross-core communication).

The host permutes src[b].T columns so each core's own query tokens are
columns 0..511 (attention contracts over all keys, so key order is
irrelevant); this keeps the SPMD program identical across cores.

Precision: all projections run bf16 x bf16 (PSUM accumulation is fp32);
the low-rank sparse-score matmul stays f32r. psp/exp values, the spmm and
the LN2 residual are bf16. Measured rel err ~4.3e-3 vs the 2e-2 gate.

Host-side folds (exact math): projection biases of the v/vsp paths commute
through the (normalized) attention rows into one residual bias row;
sigmoid(lam) is precomputed into the column-bias pack; ln1's affine is
folded into the ff1 weights; all row-broadcast constants are shipped
pre-replicated (no on-device partition broadcasts on the critical path).

Schedule highlights:
- per-head-pair score matmuls target PE row-groups 0:64/64:128 and overlap;
  both land in one 2-bank PSUM tile so a single N=1024 exp evacuates them;
- v/vsp projections computed token-major (stationary = xT tile) - no PE
  transposes; k/q proj PSUM evacuation on DVE, off the exp-saturated ACT;
- the 10-iter top-k threshold bisection (DVE) overlaps dense-path PE work;
- out_proj + spmm + fuse + LN1 run as a per-query-tile pipeline; the
  LN1 output transpose for ff1 uses XBAR DMA-transposes (no PE/DVE cost);
- ff2 streams in two column-half passes; the xg residual is folded into
  PSUM via an identity matmul and LN2 finishes per qt inside the second
  pass, overlapping the tail.
"""
import sys
import os
import contextlib

for _p in ('/opt/trn_rl_repo',):
    if _p not in sys.path:
        sys.path.insert(0, _p)

import numpy as np
import concourse.bacc as bacc
import concourse.tile as tile
from concourse import mybir
from concourse.bass_utils import run_bass_kernel_spmd
from concourse.masks import make_identity

F32 = mybir.dt.float32
F32R = mybir.dt.float32r
BF16 = mybir.dt.bfloat16
AF = mybir.ActivationFunctionType
OP = mybir.AluOpType

B, S, D, H, R, DFF = 4, 1024, 1024, 16, 64, 4096
DH = D // H          # 64
SQ = S // 2          # 512 own queries per core
KK = max(1, int(S * 0.2))   # 204
KC = D // 128        # 8 contraction chunks over D
FC = DFF // 128      # 32 chunks over DFF
NQT = SQ // 128      # 4 query tiles
NTOK = S // 128      # 8 token tiles
BISECT_ITERS = 10
INV_SQRT = 0.125     # 1/sqrt(DH) == 1/sqrt(R)

_cached = {}


def _build():
    nc = bacc.Bacc()

    def din(name, shape):
        return nc.declare_dram_parameter(name, list(shape), F32, isOutput=False)

    xT = nc.declare_dram_parameter("xT", [D, S], BF16, isOutput=False)
    x_own = din("x_own", [SQ, D])   # own rows, token-major (f32 residual)
    wqkvT = nc.declare_dram_parameter("wqkvT", [D, 3 * D], BF16,
                                      isOutput=False)
    woT = nc.declare_dram_parameter("woT", [D, D], BF16, isOutput=False)
    vpT = nc.declare_dram_parameter("vpT", [D, D], BF16, isOutput=False)
    qkpT = nc.declare_dram_parameter("qkpT", [D, 2 * R], BF16,
                                     isOutput=False)
    f1T = nc.declare_dram_parameter("f1T", [D, DFF], BF16, isOutput=False)
    f2T = nc.declare_dram_parameter("f2T", [DFF, D], BF16, isOutput=False)
    # column-bias pack (host-side layout, one contiguous DMA):
    # [:, 0:24]=in_proj_b  [:, 24:32]=Vp_b  [:, 32:64]=ff1_b
    # [:, 64:72]=ln1_g  [:, 72:80]=ln1_b  [0:64, 80]=Qp_b  [0:64, 81]=Kp_b
    bias_pack = din("bias_pack", [128, 84])
    # host-replicated row constants: 0=residual bias row, 1=b2+ln1_b,
    # 2=ln1_g, 3=ln2_g, 4=ln2_b
    rows_pack = din("rows_pack", [128, 5 * D])
    out = nc.declare_dram_parameter("out", [SQ, D], F32, isOutput=True)
    DBG = bool(os.environ.get("BASSK_DEBUG"))
    if DBG:
        dbg_dense = nc.declare_dram_parameter("dbg_dense", [SQ, D], F32,
                                              isOutput=True)
        dbg_sparse = nc.declare_dram_parameter("dbg_sparse", [SQ, D], F32,
                                               isOutput=True)
        dbg_lo = nc.declare_dram_parameter("dbg_lo", [128, NQT], F32,
                                           isOutput=True)
        dbg_rs = nc.declare_dram_parameter("dbg_rs", [128, NQT], F32,
                                           isOutput=True)

    xT_r = xT.ap().rearrange("(kc p) s -> p kc s", p=128)
    wqkvT_r = wqkvT.ap().rearrange("(kc p) f -> p kc f", p=128)
    woT_r = woT.ap().rearrange("(kc p) f -> p kc f", p=128)
    vpT_r = vpT.ap().rearrange("(kc p) f -> p kc f", p=128)
    qkpT_r = qkpT.ap().rearrange("(kc p) f -> p kc f", p=128)
    f1T_r = f1T.ap().rearrange("(kc p) f -> p kc f", p=128)
    f2T_r = f2T.ap().rearrange("(kc p) f -> p kc f", p=128)

    with tile.TileContext(nc) as tc:
        est = contextlib.ExitStack()
        with est:
            # ---------------- constants ----------------
            consts = est.enter_context(tc.tile_pool(name="consts", bufs=1))

            ident_f = consts.tile([128, 128], F32, name="ident_f")
            make_identity(nc, ident_f)
            ident_b = consts.tile([128, 128], BF16, name="ident_b")
            nc.vector.tensor_copy(out=ident_b, in_=ident_f)

            eps_t = consts.tile([128, 1], F32, name="eps_t")
            nc.vector.memset(eps_t, 1e-5)
            ones1 = consts.tile([128, 1], F32, name="ones1")
            nc.vector.memset(ones1, 1.0)
            ones16 = consts.tile([128, 16], F32, name="ones16")
            nc.vector.memset(ones16, 1.0)

            bp = consts.tile([128, 84], F32, name="bias_pack")
            bqkv_c = bp[:, 0:24]
            bvp_c = bp[:, 24:32]
            b1_c = bp[:, 32:64]
            g1_c = bp[:, 64:72]
            be1_c = bp[:, 72:80]
            bqp_c = bp[0:64, 80:81]
            bkp_c = bp[0:64, 81:82]
            sig_bc = bp[:, 82:83]
            oms_bc = bp[:, 83:84]
            ones1b = consts.tile([128, 1], BF16, name="ones1b")
            nc.vector.memset(ones1b, 1.0)
            sd_pre = consts.tile([1, 1], F32, name="sd_pre")

            def load_bias_cols():
                # one contiguous transfer on the (otherwise idle) Pool ring
                nc.gpsimd.dma_start(out=bp, in_=bias_pack.ap())


            # own-token residual (+ sig*bo); loaded after xT is in flight
            xot_pool = est.enter_context(tc.tile_pool(name="xot_pool",
                                                      bufs=1))
            xot = xot_pool.tile([128, NQT, D], F32, name="xot")

            bis = est.enter_context(tc.tile_pool(name="bis", bufs=1))
            lo = bis.tile([128, NQT], F32, name="lo")
            hi = bis.tile([128, NQT], F32, name="hi")
            mid = bis.tile([128, NQT], F32, name="mid")
            cnts = bis.tile([128, NQT], F32, name="cnts")
            pred = bis.tile([128, NQT], mybir.dt.uint32, name="pred")
            rs_sp = bis.tile([128, NQT], F32, name="rs_sp")
            rcp_sp = bis.tile([128, NQT], F32, name="rcp_sp")

            # long-lived activation groups (left stack)
            sp_stack = contextlib.ExitStack()
            sp_pool = sp_stack.enter_context(
                tc.tile_pool(name="sp_pool", bufs=1))
            Vsp = sp_pool.tile([128, NTOK, D], BF16, name="Vsp")
            kspT = sp_pool.tile([64, S], F32R, name="kspT")
            qspT = sp_pool.tile([64, SQ], F32R, name="qspT")

            wo_full = sp_pool.tile([128, KC, D], BF16, name="wo_full")

            dn_stack = contextlib.ExitStack()
            dn_pool = dn_stack.enter_context(
                tc.tile_pool(name="dn_pool", bufs=1))
            kT = dn_pool.tile([128, KC, S], BF16, name="kT")
            Vaug = dn_pool.tile([128, NTOK, H * (DH + 1)], BF16, name="Vaug")
            qT = dn_pool.tile([128, KC, SQ], BF16, name="qT")

            Vaug_h = Vaug.rearrange("p t (h c) -> p t h c", c=DH + 1)
            for t in range(NTOK):
                nc.vector.tensor_copy(out=Vaug_h[:, t, :, DH:DH + 1],
                                      in_=ones16)

            # right-stack pools (all close together after phase 7)
            psp_stack = contextlib.ExitStack()
            psp_pool = psp_stack.enter_context(
                tc.tile_pool(name="psp_pool", bufs=1, side="right"))
            psp = [psp_pool.tile([128, S], BF16, name=f"psp{qt}")
                   for qt in range(NQT)]
            scr_stack = contextlib.ExitStack()
            scr_pool = scr_stack.enter_context(
                tc.tile_pool(name="scr", bufs=1, side="right"))
            ctx_stack = contextlib.ExitStack()
            ctx_pool = ctx_stack.enter_context(
                tc.tile_pool(name="ctx_pool", bufs=1, side="right"))
            ctxT = ctx_pool.tile([128, KC, SQ], BF16, name="ctxT")
            # ============ projections + sparse path + attention ============
            with contextlib.ExitStack() as ph0:
                xt_pool = ph0.enter_context(
                    tc.tile_pool(name="xt_pool", bufs=1))
                wstr = ph0.enter_context(tc.tile_pool(name="wstr", bufs=8))
                pt_pool = ph0.enter_context(
                    tc.tile_pool(name="pt_pool", bufs=4))
                rc_pool = ph0.enter_context(
                    tc.tile_pool(name="rc_pool", bufs=1))
                ps_a = ph0.enter_context(
                    tc.tile_pool(name="ps_a", bufs=2, space="PSUM"))
                ps_b = ph0.enter_context(
                    tc.tile_pool(name="ps_b", bufs=4, space="PSUM"))

                # small sparse weights first, then xT on both queues
                qkpt = wstr.tile([128, KC, 2 * R], BF16, name="qkpt",
                                 tag="wsmall")
                nc.sync.dma_start(out=qkpt, in_=qkpT_r)
                qpt = qkpt[:, :, 0:R]
                kpt = qkpt[:, :, R:2 * R]
                xTt = xt_pool.tile([128, KC, S], BF16, name="xTt")
                for kc2 in range(4):
                    eng = nc.scalar if kc2 % 2 == 0 else nc.sync
                    eng.dma_start(out=xTt[:, 2 * kc2:2 * kc2 + 2, :],
                                  in_=xT_r[:, 2 * kc2:2 * kc2 + 2, :])
                load_bias_cols()
                vw_stack = contextlib.ExitStack()
                vw_pool = vw_stack.enter_context(
                    tc.tile_pool(name="vw_pool", bufs=2))

                # ---- sparse projections + scores ----
                with nc.named_scope("p0_ksp_qsp"):
                    ka = ps_a.tile([128, 1024], F32, name="ksa", tag="psa")
                    for nh in range(2):
                        for kc in range(KC):
                            nc.tensor.matmul(
                                ka[0:64, nh * 512:nh * 512 + 512],
                                kpt[:, kc, :],
                                xTt[:, kc, nh * 512:nh * 512 + 512],
                                start=(kc == 0), stop=(kc == KC - 1))
                    nc.scalar.activation(
                        out=kspT, in_=ka[0:64, :], func=AF.Identity,
                        bias=bkp_c, scale=1.0)
                    ps = ps_b.tile([128, 512], F32, name="ps", tag="psb")
                    for kc in range(KC):
                        nc.tensor.matmul(ps[0:64, :], qpt[:, kc, :],
                                         xTt[:, kc, 0:SQ],
                                         start=(kc == 0), stop=(kc == KC - 1))
                    nc.scalar.activation(out=qspT, in_=ps[0:64, :],
                                         func=AF.Identity, bias=bqp_c,
                                         scale=1.0)

                with nc.named_scope("p2_ssp"):
                    for qt in range(NQT):
                        ps2 = ps_a.tile([128, 1024], F32, name="ps2",
                                        tag="psa")
                        for nh in range(2):
                            nc.tensor.matmul(
                                ps2[:, nh * 512:nh * 512 + 512],
                                qspT[:, qt * 128:qt * 128 + 128],
                                kspT[:, nh * 512:nh * 512 + 512],
                                start=True, stop=True)
                        nc.scalar.activation(
                            out=psp[qt], in_=ps2, func=AF.Exp,
                            scale=INV_SQRT)

                # ---- top-k threshold bisection (DVE; overlaps PE below) ----
                with nc.named_scope("p3_bisect"):
                    nc.vector.memset(lo, 0.0)
                    nc.vector.memset(hi, 16.0)
                    for it in range(BISECT_ITERS):
                        nc.vector.tensor_add(mid, lo, hi)
                        nc.vector.tensor_scalar_mul(mid, mid, 0.5)
                        for qt in range(NQT):
                            scr = scr_pool.tile([128, S], BF16, name="scr",
                                                tag="scr")
                            nc.vector.scalar_tensor_tensor(
                                out=scr, in0=psp[qt],
                                scalar=mid[:, qt:qt + 1],
                                in1=ones1b.to_broadcast([128, S]),
                                op0=OP.is_ge, op1=OP.mult,
                                accum_out=cnts[:, qt:qt + 1])
                        nc.vector.tensor_scalar(out=pred, in0=cnts,
                                                scalar1=float(KK),
                                                scalar2=None, op0=OP.is_ge)
                        nc.vector.copy_predicated(lo, pred, mid)
                        nc.vector.tensor_scalar(out=pred, in0=cnts,
                                                scalar1=float(KK),
                                                scalar2=None, op0=OP.is_lt)
                        nc.vector.copy_predicated(hi, pred, mid)
                    for qt in range(NQT):
                        nc.vector.scalar_tensor_tensor(
                            out=psp[qt], in0=psp[qt],
                            scalar=lo[:, qt:qt + 1],
                            in1=psp[qt], op0=OP.is_ge, op1=OP.mult,
                            accum_out=rs_sp[:, qt:qt + 1])
                    if DBG:
                        nc.sync.dma_start(out=dbg_lo.ap(), in_=lo)
                        nc.sync.dma_start(out=dbg_rs.ap(), in_=rs_sp)
                    nc.vector.tensor_scalar(out=rs_sp, in0=rs_sp,
                                            scalar1=1e-9, scalar2=None,
                                            op0=OP.add)
                    nc.vector.reciprocal(rcp_sp, rs_sp)
                    nc.vector.tensor_scalar_mul(rcp_sp, rcp_sp, oms_bc)

                for qt in range(NQT):
                    nc.gpsimd.dma_start(
                        out=xot[:, qt, :],
                        in_=x_own.ap()[qt * 128:qt * 128 + 128, :])

                _wc_cnt = [0]

                def w_chunk(w_view, f0, nfs=128):
                    wt = wstr.tile([128, KC, 128], BF16, name="wt", tag="wt")
                    eng = nc.sync if _wc_cnt[0] % 2 == 0 else nc.scalar
                    _wc_cnt[0] += 1
                    eng.dma_start(out=wt[:, :, :nfs],
                                  in_=w_view[:, :, f0:f0 + nfs])
                    return wt

                # ---- v / vsp projections, directly token-major ----
                # stationary = xT token-tile chunk, moving = weight rows, so
                # the PSUM result lands token-major (no transposes, no
                # copies). Projection biases are folded into the residual
                # row host-side (softmax rows sum to 1).
                def proj_tokmajor(w_view, f_lo, to_vaug, scope):
                    with nc.named_scope(scope):
                        for fh in range(2):
                            wh = vw_pool.tile([128, KC, 512], BF16,
                                              name="wh", tag="wh")
                            eng = nc.scalar if fh == 0 else nc.sync
                            eng.dma_start(
                                out=wh,
                                in_=w_view[:, :,
                                           f_lo + fh * 512:
                                           f_lo + fh * 512 + 512])
                            for t in range(NTOK):
                                ps = ps_b.tile([128, 512], F32, name="psv",
                                               tag="psb")
                                for kc in range(KC):
                                    nc.tensor.matmul(
                                        ps,
                                        xTt[:, kc, t * 128:t * 128 + 128],
                                        wh[:, kc, :],
                                        start=(kc == 0), stop=(kc == KC - 1))
                                if to_vaug:
                                    nc.scalar.activation(
                                        out=Vaug_h[:, t, 8 * fh:8 * fh + 8,
                                                   0:DH],
                                        in_=ps, func=AF.Identity, scale=1.0)
                                else:
                                    nc.scalar.activation(
                                        out=Vsp[:, t,
                                                fh * 512:fh * 512 + 512],
                                        in_=ps, func=AF.Identity, scale=1.0)

                proj_tokmajor(wqkvT_r, 2 * D, True, "p0_v")
                proj_tokmajor(vpT_r, 0, False, "p0_vsp")
                vw_stack.close()

                # out_proj weights (needed only after attention)
                nc.scalar.dma_start(out=wo_full, in_=woT_r)

                # ---- interleaved k/q projections + dense attention ----
                with nc.named_scope("p4_kq_attn"):
                    for jj in range(4):
                        for fi in range(2):
                            ft = jj * 2 + fi
                            wkc = w_chunk(wqkvT_r, D + ft * 128)
                            for nh in range(2):
                                ps = ps_b.tile([128, 512], F32, name="ps",
                                               tag="psb")
                                for kc in range(KC):
                                    nc.tensor.matmul(
                                        ps,
                                        wkc[:, kc, 0:128],
                                        xTt[:, kc, nh * 512:nh * 512 + 512],
                                        start=(kc == 0), stop=(kc == KC - 1))
                                nc.vector.tensor_scalar(
                                    out=kT[:, ft, nh * 512:nh * 512 + 512],
                                    in0=ps,
                                    scalar1=bqkv_c[:, 8 + ft:8 + ft + 1],
                                    scalar2=None, op0=OP.add)
                            wqc = w_chunk(wqkvT_r, ft * 128)
                            ps = ps_b.tile([128, 512], F32, name="ps",
                                           tag="psb")
                            for kc in range(KC):
                                nc.tensor.matmul(
                                    ps, wqc[:, kc, 0:128],
                                    xTt[:, kc, 0:SQ],
                                    start=(kc == 0), stop=(kc == KC - 1))
                            nc.vector.tensor_scalar(
                                out=qT[:, ft, :], in0=ps,
                                scalar1=bqkv_c[:, ft:ft + 1],
                                scalar2=None, op0=OP.add)
                        # attention for the 4 heads of these two f-tiles.
                        # The two heads of an f-tile sit on partition rows
                        # 0:64 / 64:128, so their score matmuls target
                        # different PE row-groups and overlap when issued
                        # back-to-back; both land in one 2-bank PSUM tile so
                        # a single N=1024 exp evacuates the pair.
                        for ft in (2 * jj, 2 * jj + 1):
                            pctx = {po: ps_b.tile([128, 512], F32,
                                                  name="ps_c", tag="psb")
                                    for po in (0, 64)}
                            for t in range(NTOK):
                                ps2 = ps_a.tile([128, 1024], F32,
                                                name="ps_s", tag="psa")
                                for po in (0, 64):
                                    nc.tensor.matmul(
                                        ps2[:, 8 * po:8 * po + 512],
                                        kT[po:po + 64, ft,
                                           t * 128:t * 128 + 128],
                                        qT[po:po + 64, ft, :],
                                        start=True, stop=True)
                                pt = pt_pool.tile([128, 1024], BF16,
                                                  name="pT", tag="pT")
                                nc.scalar.activation(out=pt, in_=ps2,
                                                     func=AF.Exp,
                                                     scale=INV_SQRT)
                                for po in (0, 64):
                                    hh = 2 * ft + po // 64
                                    nc.tensor.matmul(
                                        pctx[po][0:65, :],
                                        Vaug[:, t, hh * 65:hh * 65 + 65],
                                        pt[:, 8 * po:8 * po + 512],
                                        start=(t == 0),
                                        stop=(t == NTOK - 1))
                            for po in (0, 64):
                                rsr = rc_pool.tile([1, 512], F32, name="rsr",
                                                   tag="rsr")
                                nc.vector.tensor_copy(out=rsr,
                                                      in_=pctx[po][64:65, :])
                                rch = rc_pool.tile([1, 512], F32, name="rch",
                                                   tag="rch")
                                nc.vector.reciprocal_approx_fast(out=rch,
                                                                 in_=rsr)
                                rb = rc_pool.tile([64, 512], F32, name="rb",
                                                  tag="rb")
                                nc.gpsimd.partition_broadcast(rb, rch)
                                nc.vector.tensor_mul(
                                    out=ctxT[po:po + 64, ft, :],
                                    in0=pctx[po][0:64, :], in1=rb)

                # pull the exp->sqrt ACT table switch off the LN1 chain
                nc.scalar.activation(out=sd_pre, in_=eps_t[0:1, :],
                                     func=AF.Sqrt, bias=eps_t[0:1, :],
                                     scale=1.0)

            dn_stack.close()   # free kT, Vaug, qT

            ds_stack = contextlib.ExitStack()
            ds_pool = ds_stack.enter_context(
                tc.tile_pool(name="ds_pool", bufs=1, side="right"))
            dense_s = ds_pool.tile([128, NQT, D], F32, name="dense_s")
            sparse_s = ds_pool.tile([128, NQT, D], F32, name="sparse_s")

            # ---- host-replicated row constants (one DMA on the idle
            # sync ring; lands mid-attention) ----
            rows_t = ds_pool.tile([128, 5, D], F32, name="rows_t")
            nc.sync.dma_start(
                out=rows_t,
                in_=rows_pack.ap().rearrange("p (r d) -> p r d", r=5))
            b12_bc = rows_t[:, 1, :]
            g1_bc = rows_t[:, 2, :]
            g2_bc = rows_t[:, 3, :]
            be2_bc = rows_t[:, 4, :]

            xg = ds_pool.tile([128, NQT, D], BF16, name="xg")
            stats = ds_pool.tile([128, NQT, 2, 6], F32, name="stats")
            mv2 = ds_pool.tile([128, NQT, 2], F32, name="mv2")
            sd = ds_pool.tile([128, NQT], F32, name="sd")
            rstd = ds_pool.tile([128, NQT], F32, name="rstd")
            x1s = ds_pool.tile([128, 2, D], F32, name="x1s")
            xhat_bf = ds_pool.tile([128, NQT, D], BF16, name="xhat_bf")

            def ln_normalize(x1, qt, out=None):
                for half in range(2):
                    nc.vector.bn_stats(
                        out=stats[:, qt, half, :],
                        in_=x1[:, half * 512:half * 512 + 512])
                nc.vector.bn_aggr(out=mv2[:, qt, :], in_=stats[:, qt])
                nc.scalar.activation(out=sd[:, qt:qt + 1],
                                     in_=mv2[:, qt, 1:2], func=AF.Sqrt,
                                     bias=eps_t, scale=1.0)
                nc.vector.reciprocal(rstd[:, qt:qt + 1], sd[:, qt:qt + 1])
                nc.vector.tensor_scalar(out=x1 if out is None else out,
                                        in0=x1,
                                        scalar1=mv2[:, qt, 0:1],
                                        scalar2=rstd[:, qt:qt + 1],
                                        op0=OP.subtract, op1=OP.mult)

            xln_stack = contextlib.ExitStack()
            xln_pool = xln_stack.enter_context(
                tc.tile_pool(name="xln_pool", bufs=1, side="right"))
            xlnT = xln_pool.tile([128, KC, SQ], BF16, name="xlnT")

            # ====== per-qt pipeline: out_proj + spmm + fuse + LN1 + xT ======
            with contextlib.ExitStack() as ph5:
                pm_pool = ph5.enter_context(
                    tc.tile_pool(name="pm_pool", bufs=1))
                ps_tr2 = ph5.enter_context(
                    tc.tile_pool(name="ps_tr2", bufs=2, space="PSUM"))
                ps_mm = ph5.enter_context(
                    tc.tile_pool(name="ps_mm", bufs=6, space="PSUM"))
                pmT = pm_pool.tile([128, NTOK, SQ], BF16, name="pmT")
                with nc.named_scope("p5_outproj"):
                    # all masked-p transposes first: they depend only on
                    # psp, so the PE stays busy while the attention tail's
                    # DVE normalize chain drains
                    for qt in range(NQT):
                        for t in range(NTOK):
                            pst = ps_tr2.tile([128, 128], BF16, name="pst2",
                                              tag="pst2")
                            nc.tensor.transpose(
                                pst, psp[qt][:, t * 128:t * 128 + 128],
                                ident_b)
                            nc.scalar.copy(
                                out=pmT[:, t, qt * 128:qt * 128 + 128],
                                in_=pst)
                    for qt in range(NQT):
                        sps = []
                        for nh in range(2):
                            ps = ps_mm.tile([128, 512], F32, name="ps_o",
                                            tag="ps_o")
                            for t in range(NTOK):
                                nc.tensor.matmul(
                                    ps,
                                    pmT[:, t, qt * 128:qt * 128 + 128],
                                    Vsp[:, t, nh * 512:nh * 512 + 512],
                                    start=(t == 0), stop=(t == NTOK - 1))
                            sps.append(ps)
                        # dense out_proj for this query tile
                        pss2 = [ps_mm.tile([128, 512], F32, name="ps_o",
                                           tag="ps_o") for _ in range(2)]
                        for kc in range(KC):
                            for nh in range(2):
                                nc.tensor.matmul(
                                    pss2[nh],
                                    ctxT[:, kc, qt * 128:qt * 128 + 128],
                                    wo_full[:, kc, nh * 512:nh * 512 + 512],
                                    start=(kc == 0), stop=(kc == KC - 1))
                        # fuse directly from PSUM on DVE:
                        #   x1 = sparse_ps*rcp + xot, then += dense_ps*sig
                        x1 = x1s[:, qt % 2, :]
                        for nh in range(2):
                            sl = slice(nh * 512, nh * 512 + 512)
                            nc.vector.scalar_tensor_tensor(
                                out=x1[:, sl], in0=sps[nh],
                                scalar=rcp_sp[:, qt:qt + 1],
                                in1=xot[:, qt, sl],
                                op0=OP.mult, op1=OP.add)
                        for nh in range(2):
                            sl = slice(nh * 512, nh * 512 + 512)
                            nc.vector.scalar_tensor_tensor(
                                out=x1[:, sl], in0=pss2[nh],
                                scalar=sig_bc,
                                in1=x1[:, sl],
                                op0=OP.mult, op1=OP.add)
                        if DBG:
                            for nh in range(2):
                                sl = slice(nh * 512, nh * 512 + 512)
                                nc.scalar.activation(
                                    out=sparse_s[:, qt, sl], in_=sps[nh],
                                    func=AF.Copy, scale=rcp_sp[:, qt:qt + 1])
                                nc.scalar.activation(
                                    out=dense_s[:, qt, sl], in_=pss2[nh],
                                    func=AF.Copy, scale=sig_bc)
                        ln_normalize(x1, qt, out=xhat_bf[:, qt, :])
                        nc.sync.dma_start_transpose(
                            out=xlnT[:, :, qt * 128:qt * 128 + 128],
                            in_=xhat_bf[:, qt, :])
            if DBG:
                for qt in range(NQT):
                    nc.sync.dma_start(
                        out=dbg_dense.ap()[qt * 128:qt * 128 + 128, :],
                        in_=dense_s[:, qt, :])
                    nc.sync.dma_start(
                        out=dbg_sparse.ap()[qt * 128:qt * 128 + 128, :],
                        in_=sparse_s[:, qt, :])
            sp_stack.close()

            # xg (LN2 residual) on DVE while ff1 owns the PE
            for qt in range(NQT):
                nc.vector.tensor_mul(xg[:, qt, :], xhat_bf[:, qt, :], g1_bc)
                nc.vector.tensor_add(xg[:, qt, :], xg[:, qt, :], b12_bc)

            # ============ ff1 + relu ============
            h1_stack = contextlib.ExitStack()
            h1_pool = h1_stack.enter_context(
                tc.tile_pool(name="h1_pool", bufs=1))
            h1T = h1_pool.tile([128, FC, SQ], BF16, name="h1T")
            with contextlib.ExitStack() as ph9:
                w3str = ph9.enter_context(tc.tile_pool(name="w3str", bufs=4))
                ps_f1 = ph9.enter_context(
                    tc.tile_pool(name="ps_f1", bufs=4, space="PSUM"))
                with nc.named_scope("p9_ff1"):
                    for jj in range(16):
                        wt = w3str.tile([128, KC, 256], BF16, name="w1t",
                                        tag="w3")
                        f0 = jj * 256
                        eng = nc.scalar if jj % 2 == 0 else nc.sync
                        eng.dma_start(out=wt, in_=f1T_r[:, :, f0:f0 + 256])
                        for fi in range(2):
                            dft = jj * 2 + fi
                            ps = ps_f1.tile([128, 512], F32, name="ps_f",
                                            tag="ps_f")
                            for kc in range(KC):
                                nc.tensor.matmul(
                                    ps, wt[:, kc, fi * 128:fi * 128 + 128],
                                    xlnT[:, kc, :],
                                    start=(kc == 0), stop=(kc == KC - 1))
                            if jj % 2 == 0:
                                nc.scalar.activation(
                                    out=h1T[:, dft, :], in_=ps,
                                    func=AF.Relu,
                                    bias=b1_c[:, dft:dft + 1], scale=1.0)
                            else:
                                nc.vector.tensor_scalar(
                                    out=h1T[:, dft, :], in0=ps,
                                    scalar1=b1_c[:, dft:dft + 1],
                                    scalar2=0.0, op0=OP.add, op1=OP.max)
            xln_stack.close()

            # ============ ff2 + residual + LN2 + out ============
            ff_s = ds_pool.tile([128, NQT, D], F32, name="ff_s")
            with contextlib.ExitStack() as ph10:
                w4str = ph10.enter_context(tc.tile_pool(name="w4str",
                                                        bufs=8))
                w4b_pool = ph10.enter_context(
                    tc.tile_pool(name="w4b_pool", bufs=1))
                ps_f2 = ph10.enter_context(
                    tc.tile_pool(name="ps_f2", bufs=8, space="PSUM"))
                with nc.named_scope("p10_ff2"):
                    pss = [ps_f2.tile([128, 512], F32, name="ps_g",
                                      tag="ps_g") for _ in range(8)]
                    # prefetch the second column-half of f2T (resident for
                    # the qt-major second pass)
                    f2b = w4b_pool.tile([128, FC, 512], BF16, name="f2b")
                    for kc in range(FC):
                        eng = nc.scalar if kc % 2 == 0 else nc.sync
                        eng.dma_start(
                            out=f2b[:, kc, :],
                            in_=f2T_r[:, kc, 512:1024])
                    # pass 0: stream the first half kc-major
                    for kc in range(FC):
                        f2h = w4str.tile([128, 512], BF16, name="f2h",
                                         tag="w4")
                        eng = nc.scalar if kc % 2 == 0 else nc.sync
                        eng.dma_start(out=f2h,
                                      in_=f2T_r[:, kc, 0:512])
                        for qt in range(NQT):
                            nc.tensor.matmul(
                                pss[2 * qt],
                                h1T[:, kc, qt * 128:qt * 128 + 128],
                                f2h, start=(kc == 0), stop=(kc == FC - 1))
                    for qt in range(NQT):
                        nc.tensor.matmul(
                            pss[2 * qt], ident_b, xg[:, qt, 0:512],
                            start=False, stop=True)
                        nc.vector.bn_stats(out=stats[:, qt, 0, :],
                                           in_=pss[2 * qt])
                    # pass 1: qt-major on the resident half; finish LN2 and
                    # store per qt while later qt's matmuls run
                    for qt in range(NQT):
                        for kc in range(FC):
                            nc.tensor.matmul(
                                pss[2 * qt + 1],
                                h1T[:, kc, qt * 128:qt * 128 + 128],
                                f2b[:, kc, :],
                                start=(kc == 0), stop=(kc == FC - 1))
                        nc.tensor.matmul(
                            pss[2 * qt + 1], ident_b, xg[:, qt, 512:1024],
                            start=False, stop=True)
                        nc.vector.bn_stats(out=stats[:, qt, 1, :],
                                           in_=pss[2 * qt + 1])
                        nc.vector.bn_aggr(out=mv2[:, qt, :],
                                          in_=stats[:, qt])
                        nc.scalar.activation(out=sd[:, qt:qt + 1],
                                             in_=mv2[:, qt, 1:2],
                                             func=AF.Sqrt, bias=eps_t,
                                             scale=1.0)
                        nc.vector.reciprocal(rstd[:, qt:qt + 1],
                                             sd[:, qt:qt + 1])
                        x2 = ff_s[:, qt, :]
                        for half in range(2):
                            nc.vector.tensor_scalar(
                                out=x2[:, half * 512:half * 512 + 512],
                                in0=pss[2 * qt + half],
                                scalar1=mv2[:, qt, 0:1],
                                scalar2=rstd[:, qt:qt + 1],
                                op0=OP.subtract, op1=OP.mult)
                        ot = ds_pool.tile([128, D], F32, name="out_t",
                                          tag="out_t", bufs=2)
                        nc.vector.tensor_mul(ot[:, 0:512], x2[:, 0:512],
                                             g2_bc[:, 0:512])
                        nc.vector.tensor_add(ot[:, 0:512], ot[:, 0:512],
                                             be2_bc[:, 0:512])
                        nc.gpsimd.tensor_mul(ot[:, 512:1024],
                                             x2[:, 512:1024],
                                             g2_bc[:, 512:1024])
                        nc.gpsimd.tensor_add(ot[:, 512:1024],
                                             ot[:, 512:1024],
                                             be2_bc[:, 512:1024])
                        nc.sync.dma_start(
                            out=out.ap()[qt * 128:qt * 128 + 128, :],
                            in_=ot)
            h1_stack.close()
            ds_stack.close()
            ctx_stack.close()
            scr_stack.close()
            psp_stack.close()

    nc.compile()
    return nc


def _prep_inputs(src, in_proj_w, in_proj_b, out_proj_w, out_proj_b,
                 Qp_w, Qp_b, Kp_w, Kp_b, Vp_w, Vp_b, lam,
                 ff1_w, ff1_b, ff2_w, ff2_b, ln1_g, ln1_b, ln2_g, ln2_b):
    import ml_dtypes
    f = np.float32
    A = lambda x: np.ascontiguousarray(x, dtype=f)
    AB = lambda x: np.ascontiguousarray(np.asarray(x, dtype=f),
                                        dtype=ml_dtypes.bfloat16)
    bias_pack = np.zeros((128, 84), np.float32)
    bias_pack[:, 0:24] = np.asarray(in_proj_b, np.float32).reshape(24, 128).T
    bias_pack[:, 24:32] = np.asarray(Vp_b, np.float32).reshape(8, 128).T
    # ln1 affine folded into ff1: relu((x*g1+be1) @ W1.T + b1)
    #   = relu(x @ (W1*g1).T + (b1 + W1@be1))
    w1f = np.asarray(ff1_w, np.float32)
    g1f = np.asarray(ln1_g, np.float32)
    b1_eff = (np.asarray(ff1_b, np.float32)
              + w1f @ np.asarray(ln1_b, np.float32))
    bias_pack[:, 32:64] = b1_eff.reshape(32, 128).T
    bias_pack[:, 64:72] = np.asarray(ln1_g, np.float32).reshape(8, 128).T
    bias_pack[:, 72:80] = np.asarray(ln1_b, np.float32).reshape(8, 128).T
    bias_pack[0:64, 80] = np.asarray(Qp_b, np.float32)
    bias_pack[0:64, 81] = np.asarray(Kp_b, np.float32)
    sigf = 1.0 / (1.0 + np.exp(-np.float32(np.asarray(lam))))
    bias_pack[:, 82] = sigf
    bias_pack[:, 83] = 1.0 - sigf
    # residual bias row: v-projection biases commute through the
    # (normalized) attention rows, so fold them host-side:
    #   sig*(out_proj_b + v_b @ Wo.T) + (1-sig)*Vp_b
    v_b = np.asarray(in_proj_b, np.float32)[2 * D:3 * D]
    bo_row = (sigf * (np.asarray(out_proj_b, np.float32)
                      + v_b @ np.asarray(out_proj_w, np.float32).T)
              + (1.0 - sigf) * np.asarray(Vp_b, np.float32))
    rows = np.stack([
        bo_row,
        np.asarray(ff2_b, np.float32) + np.asarray(ln1_b, np.float32),
        np.asarray(ln1_g, np.float32),
        np.asarray(ln2_g, np.float32),
        np.asarray(ln2_b, np.float32),
    ]).reshape(-1)
    rows_rep = np.ascontiguousarray(
        np.broadcast_to(rows[None, :], (128, 5 * D)), np.float32)
    shared = {
        "wqkvT": AB(np.asarray(in_proj_w).T),
        "woT": AB(np.asarray(out_proj_w).T),
        "vpT": AB(np.asarray(Vp_w).T),
        "qkpT": AB(np.concatenate([np.asarray(Qp_w).T, np.asarray(Kp_w).T],
                                  axis=1)),
        "f1T": AB((w1f * g1f[None, :]).T),
        "f2T": AB(np.asarray(ff2_w).T),
        "bias_pack": A(bias_pack),
        "rows_pack": rows_rep,
    }
    in_maps = []
    for core in range(8):
        b, h = core // 2, core % 2
        srcb = np.asarray(src[b])
        xTb = srcb.T
        if h == 1:
            # own-query columns first (key order is irrelevant to attention)
            xTb = np.concatenate([xTb[:, SQ:], xTb[:, :SQ]], axis=1)
        m = dict(shared)
        m["xT"] = AB(xTb)
        m["x_own"] = A(srcb[h * SQ:(h + 1) * SQ, :] + bo_row[None, :])
        in_maps.append(m)
    return in_maps


def _run(inputs, trace=False):
    if "nc" not in _cached:
        _cached["nc"] = _build()
    nc = _cached["nc"]
    in_maps = _prep_inputs(**inputs)
    res = run_bass_kernel_spmd(nc, in_maps, core_ids=list(range(8)),
                               trace=trace)
    out = np.empty((B, S, D), np.float32)
    for core in range(8):
        b, h = core // 2, core % 2
        out[b, h * SQ:(h + 1) * SQ, :] = res.results[core]["out"]
    return out, res


def kernel(**inputs) -> np.ndarray:
    out, _ = _run(inputs, trace=False)
    return out



# revision 43
# speedup vs baseline: 1.1029x; 1.0262x over previous
"""Trainium2 Bass kernel for the EnhancedEncoderLayer (dense MHA + low-rank
top-k sparse attention + FFN, two layernorms).

Sharding: 8 cores = (batch b in 0..3) x (query-half h in {0,1}). Each core
computes output rows [b, h*512:(h+1)*512, :]. K/V-side projections are
computed redundantly per batch pair (no cross-core communication).

The host permutes src[b].T columns so each core's own query tokens are
columns 0..511 (attention contracts over all keys, so key order is
irrelevant); this keeps the SPMD program identical across cores.

Precision: all projections run bf16 x bf16 (PSUM accumulation is fp32);
the low-rank sparse-score matmul stays f32r. psp/exp values, the spmm and
the LN2 residual are bf16. Measured rel err ~4.3e-3 vs the 2e-2 gate.

Host-side folds (exact math): projection biases of the v/vsp paths commute
through the (normalized) attention rows into one residual bias row;
sigmoid(lam) is precomputed into the column-bias pack; ln1's affine is
folded into the ff1 weights; all row-broadcast constants are shipped
pre-replicated (no on-device partition broadcasts on the critical path).

Schedule highlights:
- per-head-pair score matmuls target PE row-groups 0:64/64:128 and overlap;
  both land in one 2-bank PSUM tile so a single N=1024 exp evacuates them;
- v/vsp projections computed token-major (stationary = xT tile) - no PE
  transposes; k/q proj PSUM evacuation on DVE, off the exp-saturated ACT;
- the 10-iter top-k threshold bisection (DVE) overlaps dense-path PE work;
- out_proj + spmm + fuse + LN1 run as a per-query-tile pipeline; the
  LN1 output transpose for ff1 uses XBAR DMA-transposes (no PE/DVE cost);
- ff2 streams in two column-half passes; the xg residual is folded into
  PSUM via an identity matmul and LN2 finishes per qt inside the second
  pass, overlapping the tail.
"""
import sys
import os
import contextlib

for _p in ('/opt/trn_rl_repo',):
    if _p not in sys.path:
        sys.path.insert(0, _p)

import numpy as np
import concourse.bacc as bacc
import concourse.tile as tile
from concourse import mybir
from concourse.bass_utils import run_bass_kernel_spmd
from concourse.masks import make_identity

F32 = mybir.dt.float32
F32R = mybir.dt.float32r
BF16 = mybir.dt.bfloat16
AF = mybir.ActivationFunctionType
OP = mybir.AluOpType

B, S, D, H, R, DFF = 4, 1024, 1024, 16, 64, 4096
DH = D // H          # 64
SQ = S // 2          # 512 own queries per core
KK = max(1, int(S * 0.2))   # 204
KC = D // 128        # 8 contraction chunks over D
FC = DFF // 128      # 32 chunks over DFF
NQT = SQ // 128      # 4 query tiles
NTOK = S // 128      # 8 token tiles
BISECT_ITERS = 10
INV_SQRT = 0.125     # 1/sqrt(DH) == 1/sqrt(R)

_cached = {}


def _build():
    nc = bacc.Bacc()

    def din(name, shape):
        return nc.declare_dram_parameter(name, list(shape), F32, isOutput=False)

    xT = nc.declare_dram_parameter("xT", [D, S], BF16, isOutput=False)
    x_own = din("x_own", [SQ, D])   # own rows, token-major (f32 residual)
    wqkvT = nc.declare_dram_parameter("wqkvT", [D, 3 * D], BF16,
                                      isOutput=False)
    woT = nc.declare_dram_parameter("woT", [D, D], BF16, isOutput=False)
    vpT = nc.declare_dram_parameter("vpT", [D, D], BF16, isOutput=False)
    qkpT = nc.declare_dram_parameter("qkpT", [D, 2 * R], BF16,
                                     isOutput=False)
    f1T = nc.declare_dram_parameter("f1T", [D, DFF], BF16, isOutput=False)
    f2T = nc.declare_dram_parameter("f2T", [DFF, D], BF16, isOutput=False)
    # column-bias pack (host-side layout, one contiguous DMA):
    # [:, 0:24]=in_proj_b  [:, 24:32]=Vp_b  [:, 32:64]=ff1_b
    # [:, 64:72]=ln1_g  [:, 72:80]=ln1_b  [0:64, 80]=Qp_b  [0:64, 81]=Kp_b
    bias_pack = din("bias_pack", [128, 84])
    # host-replicated row constants: 0=residual bias row, 1=b2+ln1_b,
    # 2=ln1_g, 3=ln2_g, 4=ln2_b
    rows_pack = din("rows_pack", [128, 5 * D])
    out = nc.declare_dram_parameter("out", [SQ, D], F32, isOutput=True)
    DBG = bool(os.environ.get("BASSK_DEBUG"))
    if DBG:
        dbg_dense = nc.declare_dram_parameter("dbg_dense", [SQ, D], F32,
                                              isOutput=True)
        dbg_sparse = nc.declare_dram_parameter("dbg_sparse", [SQ, D], F32,
                                               isOutput=True)
        dbg_lo = nc.declare_dram_parameter("dbg_lo", [128, NQT], F32,
                                           isOutput=True)
        dbg_rs = nc.declare_dram_parameter("dbg_rs", [128, NQT], F32,
                                           isOutput=True)

    xT_r = xT.ap().rearrange("(kc p) s -> p kc s", p=128)
    wqkvT_r = wqkvT.ap().rearrange("(kc p) f -> p kc f", p=128)
    woT_r = woT.ap().rearrange("(kc p) f -> p kc f", p=128)
    vpT_r = vpT.ap().rearrange("(kc p) f -> p kc f", p=128)
    qkpT_r = qkpT.ap().rearrange("(kc p) f -> p kc f", p=128)
    f1T_r = f1T.ap().rearrange("(kc p) f -> p kc f", p=128)
    f2T_r = f2T.ap().rearrange("(kc p) f -> p kc f", p=128)

    with tile.TileContext(nc) as tc:
        est = contextlib.ExitStack()
        with est:
            # ---------------- constants ----------------
            consts = est.enter_context(tc.tile_pool(name="consts", bufs=1))

            ident_f = consts.tile([128, 128], F32, name="ident_f")
            make_identity(nc, ident_f)
            ident_b = consts.tile([128, 128], BF16, name="ident_b")
            nc.vector.tensor_copy(out=ident_b, in_=ident_f)

            eps_t = consts.tile([128, 1], F32, name="eps_t")
            nc.vector.memset(eps_t, 1e-5)
            ones1 = consts.tile([128, 1], F32, name="ones1")
            nc.vector.memset(ones1, 1.0)
            ones16 = consts.tile([128, 16], F32, name="ones16")
            nc.vector.memset(ones16, 1.0)

            bp = consts.tile([128, 84], F32, name="bias_pack")
            bqkv_c = bp[:, 0:24]
            bvp_c = bp[:, 24:32]
            b1_c = bp[:, 32:64]
            g1_c = bp[:, 64:72]
            be1_c = bp[:, 72:80]
            bqp_c = bp[0:64, 80:81]
            bkp_c = bp[0:64, 81:82]
            sig_bc = bp[:, 82:83]
            oms_bc = bp[:, 83:84]
            ones1b = consts.tile([128, 1], BF16, name="ones1b")
            nc.vector.memset(ones1b, 1.0)
            sd_pre = consts.tile([1, 1], F32, name="sd_pre")

            def load_bias_cols():
                # one contiguous transfer on the (otherwise idle) Pool ring
                nc.gpsimd.dma_start(out=bp, in_=bias_pack.ap())


            # own-token residual (+ sig*bo); loaded after xT is in flight
            xot_pool = est.enter_context(tc.tile_pool(name="xot_pool",
                                                      bufs=1))
            xot = xot_pool.tile([128, NQT, D], F32, name="xot")

            bis = est.enter_context(tc.tile_pool(name="bis", bufs=1))
            lo = bis.tile([128, NQT], F32, name="lo")
            hi = bis.tile([128, NQT], F32, name="hi")
            mid = bis.tile([128, NQT], F32, name="mid")
            cnts = bis.tile([128, NQT], F32, name="cnts")
            pred = bis.tile([128, NQT], mybir.dt.uint32, name="pred")
            rs_sp = bis.tile([128, NQT], F32, name="rs_sp")
            rcp_sp = bis.tile([128, NQT], F32, name="rcp_sp")

            # long-lived activation groups (left stack)
            sp_stack = contextlib.ExitStack()
            sp_pool = sp_stack.enter_context(
                tc.tile_pool(name="sp_pool", bufs=1))
            Vsp = sp_pool.tile([128, NTOK, D], BF16, name="Vsp")
            kspT = sp_pool.tile([64, S], F32R, name="kspT")
            qspT = sp_pool.tile([64, SQ], F32R, name="qspT")

            wo_full = sp_pool.tile([128, KC, D], BF16, name="wo_full")

            dn_stack = contextlib.ExitStack()
            dn_pool = dn_stack.enter_context(
                tc.tile_pool(name="dn_pool", bufs=1))
            kT = dn_pool.tile([128, KC, S], BF16, name="kT")
            Vaug = dn_pool.tile([128, NTOK, H * (DH + 1)], BF16, name="Vaug")
            qT = dn_pool.tile([128, KC, SQ], BF16, name="qT")

            Vaug_h = Vaug.rearrange("p t (h c) -> p t h c", c=DH + 1)
            for t in range(NTOK):
                nc.vector.tensor_copy(out=Vaug_h[:, t, :, DH:DH + 1],
                                      in_=ones16)

            # right-stack pools (all close together after phase 7)
            psp_stack = contextlib.ExitStack()
            psp_pool = psp_stack.enter_context(
                tc.tile_pool(name="psp_pool", bufs=1, side="right"))
            psp = [psp_pool.tile([128, S], BF16, name=f"psp{qt}")
                   for qt in range(NQT)]
            scr_stack = contextlib.ExitStack()
            scr_pool = scr_stack.enter_context(
                tc.tile_pool(name="scr", bufs=1, side="right"))
            ctx_stack = contextlib.ExitStack()
            ctx_pool = ctx_stack.enter_context(
                tc.tile_pool(name="ctx_pool", bufs=1, side="right"))
            ctxT = ctx_pool.tile([128, KC, SQ], BF16, name="ctxT")
            # ============ projections + sparse path + attention ============
            with contextlib.ExitStack() as ph0:
                xt_pool = ph0.enter_context(
                    tc.tile_pool(name="xt_pool", bufs=1))
                wstr = ph0.enter_context(tc.tile_pool(name="wstr", bufs=8))
                pt_pool = ph0.enter_context(
                    tc.tile_pool(name="pt_pool", bufs=4))
                rc_pool = ph0.enter_context(
                    tc.tile_pool(name="rc_pool", bufs=1))
                ps_a = ph0.enter_context(
                    tc.tile_pool(name="ps_a", bufs=2, space="PSUM"))
                ps_b = ph0.enter_context(
                    tc.tile_pool(name="ps_b", bufs=4, space="PSUM"))

                # small sparse weights first, then xT on both queues
                qkpt = wstr.tile([128, KC, 2 * R], BF16, name="qkpt",
                                 tag="wsmall")
                nc.sync.dma_start(out=qkpt, in_=qkpT_r)
                qpt = qkpt[:, :, 0:R]
                kpt = qkpt[:, :, R:2 * R]
                xTt = xt_pool.tile([128, KC, S], BF16, name="xTt")
                for kc2 in range(4):
                    eng = nc.scalar if kc2 % 2 == 0 else nc.sync
                    eng.dma_start(out=xTt[:, 2 * kc2:2 * kc2 + 2, :],
                                  in_=xT_r[:, 2 * kc2:2 * kc2 + 2, :])
                load_bias_cols()
                vw_stack = contextlib.ExitStack()
                vw_pool = vw_stack.enter_context(
                    tc.tile_pool(name="vw_pool", bufs=2))

                # ---- sparse projections + scores ----
                with nc.named_scope("p0_ksp_qsp"):
                    ka = ps_a.tile([128, 1024], F32, name="ksa", tag="psa")
                    for nh in range(2):
                        for kc in range(KC):
                            nc.tensor.matmul(
                                ka[0:64, nh * 512:nh * 512 + 512],
                                kpt[:, kc, :],
                                xTt[:, kc, nh * 512:nh * 512 + 512],
                                start=(kc == 0), stop=(kc == KC - 1))
                    nc.scalar.activation(
                        out=kspT, in_=ka[0:64, :], func=AF.Identity,
                        bias=bkp_c, scale=1.0)
                    ps = ps_b.tile([128, 512], F32, name="ps", tag="psb")
                    for kc in range(KC):
                        nc.tensor.matmul(ps[0:64, :], qpt[:, kc, :],
                                         xTt[:, kc, 0:SQ],
                                         start=(kc == 0), stop=(kc == KC - 1))
                    nc.scalar.activation(out=qspT, in_=ps[0:64, :],
                                         func=AF.Identity, bias=bqp_c,
                                         scale=1.0)

                with nc.named_scope("p2_ssp"):
                    for qt in range(NQT):
                        ps2 = ps_a.tile([128, 1024], F32, name="ps2",
                                        tag="psa")
                        for nh in range(2):
                            nc.tensor.matmul(
                                ps2[:, nh * 512:nh * 512 + 512],
                                qspT[:, qt * 128:qt * 128 + 128],
                                kspT[:, nh * 512:nh * 512 + 512],
                                start=True, stop=True)
                        nc.scalar.activation(
                            out=psp[qt], in_=ps2, func=AF.Exp,
                            scale=INV_SQRT)

                # ---- top-k threshold bisection (DVE; overlaps PE below) ----
                with nc.named_scope("p3_bisect"):
                    nc.vector.memset(lo, 0.0)
                    nc.vector.memset(hi, 16.0)
                    for it in range(BISECT_ITERS):
                        nc.vector.tensor_add(mid, lo, hi)
                        nc.vector.tensor_scalar_mul(mid, mid, 0.5)
                        for qt in range(NQT):
                            scr = scr_pool.tile([128, S], BF16, name="scr",
                                                tag="scr")
                            nc.vector.scalar_tensor_tensor(
                                out=scr, in0=psp[qt],
                                scalar=mid[:, qt:qt + 1],
                                in1=ones1b.to_broadcast([128, S]),
                                op0=OP.is_ge, op1=OP.mult,
                                accum_out=cnts[:, qt:qt + 1])
                        nc.vector.tensor_scalar(out=pred, in0=cnts,
                                                scalar1=float(KK),
                                                scalar2=None, op0=OP.is_ge)
                        nc.vector.copy_predicated(lo, pred, mid)
                        nc.vector.tensor_scalar(out=pred, in0=cnts,
                                                scalar1=float(KK),
                                                scalar2=None, op0=OP.is_lt)
                        nc.vector.copy_predicated(hi, pred, mid)
                    for qt in range(NQT):
                        nc.vector.scalar_tensor_tensor(
                            out=psp[qt], in0=psp[qt],
                            scalar=lo[:, qt:qt + 1],
                            in1=psp[qt], op0=OP.is_ge, op1=OP.mult,
                            accum_out=rs_sp[:, qt:qt + 1])
                    if DBG:
                        nc.sync.dma_start(out=dbg_lo.ap(), in_=lo)
                        nc.sync.dma_start(out=dbg_rs.ap(), in_=rs_sp)
                    nc.vector.tensor_scalar(out=rs_sp, in0=rs_sp,
                                            scalar1=1e-9, scalar2=None,
                                            op0=OP.add)
                    nc.vector.reciprocal(rcp_sp, rs_sp)
                    nc.vector.tensor_scalar_mul(rcp_sp, rcp_sp, oms_bc)

                for qt in range(NQT):
                    nc.gpsimd.dma_start(
                        out=xot[:, qt, :],
                        in_=x_own.ap()[qt * 128:qt * 128 + 128, :])

                _wc_cnt = [0]

                def w_chunk(w_view, f0, nfs=128):
                    wt = wstr.tile([128, KC, 128], BF16, name="wt", tag="wt")
                    eng = nc.sync if _wc_cnt[0] % 2 == 0 else nc.scalar
                    _wc_cnt[0] += 1
                    eng.dma_start(out=wt[:, :, :nfs],
                                  in_=w_view[:, :, f0:f0 + nfs])
                    return wt

                # ---- v / vsp projections, directly token-major ----
                # stationary = xT token-tile chunk, moving = weight rows, so
                # the PSUM result lands token-major (no transposes, no
                # copies). Projection biases are folded into the residual
                # row host-side (softmax rows sum to 1).
                def proj_tokmajor(w_view, f_lo, to_vaug, scope):
                    with nc.named_scope(scope):
                        for fh in range(2):
                            wh = vw_pool.tile([128, KC, 512], BF16,
                                              name="wh", tag="wh")
                            eng = nc.scalar if fh == 0 else nc.sync
                            eng.dma_start(
                                out=wh,
                                in_=w_view[:, :,
                                           f_lo + fh * 512:
                                           f_lo + fh * 512 + 512])
                            for t in range(NTOK):
                                ps = ps_b.tile([128, 512], F32, name="psv",
                                               tag="psb")
                                for kc in range(KC):
                                    nc.tensor.matmul(
                                        ps,
                                        xTt[:, kc, t * 128:t * 128 + 128],
                                        wh[:, kc, :],
                                        start=(kc == 0), stop=(kc == KC - 1))
                                if to_vaug:
                                    nc.scalar.activation(
                                        out=Vaug_h[:, t, 8 * fh:8 * fh + 8,
                                                   0:DH],
                                        in_=ps, func=AF.Identity, scale=1.0)
                                else:
                                    nc.scalar.activation(
                                        out=Vsp[:, t,
                                                fh * 512:fh * 512 + 512],
                                        in_=ps, func=AF.Identity, scale=1.0)

                proj_tokmajor(wqkvT_r, 2 * D, True, "p0_v")
                proj_tokmajor(vpT_r, 0, False, "p0_vsp")
                vw_stack.close()

                # out_proj weights (needed only after attention)
                nc.scalar.dma_start(out=wo_full, in_=woT_r)

                # ---- interleaved k/q projections + dense attention ----
                with nc.named_scope("p4_kq_attn"):
                    for jj in range(4):
                        for fi in range(2):
                            ft = jj * 2 + fi
                            wkc = w_chunk(wqkvT_r, D + ft * 128)
                            for nh in range(2):
                                ps = ps_b.tile([128, 512], F32, name="ps",
                                               tag="psb")
                                for kc in range(KC):
                                    nc.tensor.matmul(
                                        ps,
                                        wkc[:, kc, 0:128],
                                        xTt[:, kc, nh * 512:nh * 512 + 512],
                                        start=(kc == 0), stop=(kc == KC - 1))
                                nc.vector.tensor_scalar(
                                    out=kT[:, ft, nh * 512:nh * 512 + 512],
                                    in0=ps,
                                    scalar1=bqkv_c[:, 8 + ft:8 + ft + 1],
                                    scalar2=None, op0=OP.add)
                            wqc = w_chunk(wqkvT_r, ft * 128)
                            ps = ps_b.tile([128, 512], F32, name="ps",
                                           tag="psb")
                            for kc in range(KC):
                                nc.tensor.matmul(
                                    ps, wqc[:, kc, 0:128],
                                    xTt[:, kc, 0:SQ],
                                    start=(kc == 0), stop=(kc == KC - 1))
                            nc.vector.tensor_scalar(
                                out=qT[:, ft, :], in0=ps,
                                scalar1=bqkv_c[:, ft:ft + 1],
                                scalar2=None, op0=OP.add)
                        # attention for the 4 heads of these two f-tiles.
                        # The two heads of an f-tile sit on partition rows
                        # 0:64 / 64:128, so their score matmuls target
                        # different PE row-groups and overlap when issued
                        # back-to-back; both land in one 2-bank PSUM tile so
                        # a single N=1024 exp evacuates the pair.
                        for ft in (2 * jj, 2 * jj + 1):
                            pctx = {po: ps_b.tile([128, 512], F32,
                                                  name="ps_c", tag="psb")
                                    for po in (0, 64)}
                            for t in range(NTOK):
                                ps2 = ps_a.tile([128, 1024], F32,
                                                name="ps_s", tag="psa")
                                for po in (0, 64):
                                    nc.tensor.matmul(
                                        ps2[:, 8 * po:8 * po + 512],
                                        kT[po:po + 64, ft,
                                           t * 128:t * 128 + 128],
                                        qT[po:po + 64, ft, :],
                                        start=True, stop=True)
                                pt = pt_pool.tile([128, 1024], BF16,
                                                  name="pT", tag="pT")
                                nc.scalar.activation(out=pt, in_=ps2,
                                                     func=AF.Exp,
                                                     scale=INV_SQRT)
                                for po in (0, 64):
                                    hh = 2 * ft + po // 64
                                    nc.tensor.matmul(
                                        pctx[po][0:65, :],
                                        Vaug[:, t, hh * 65:hh * 65 + 65],
                                        pt[:, 8 * po:8 * po + 512],
                                        start=(t == 0),
                                        stop=(t == NTOK - 1))
                            for po in (0, 64):
                                rsr = rc_pool.tile([1, 512], F32, name="rsr",
                                                   tag="rsr")
                                nc.vector.tensor_copy(out=rsr,
                                                      in_=pctx[po][64:65, :])
                                rch = rc_pool.tile([1, 512], F32, name="rch",
                                                   tag="rch")
                                nc.vector.reciprocal_approx_fast(out=rch,
                                                                 in_=rsr)
                                rb = rc_pool.tile([64, 512], F32, name="rb",
                                                  tag="rb")
                                nc.gpsimd.partition_broadcast(rb, rch)
                                nc.vector.tensor_mul(
                                    out=ctxT[po:po + 64, ft, :],
                                    in0=pctx[po][0:64, :], in1=rb)

                # pull the exp->sqrt ACT table switch off the LN1 chain
                nc.scalar.activation(out=sd_pre, in_=eps_t[0:1, :],
                                     func=AF.Sqrt, bias=eps_t[0:1, :],
                                     scale=1.0)

            dn_stack.close()   # free kT, Vaug, qT

            ds_stack = contextlib.ExitStack()
            ds_pool = ds_stack.enter_context(
                tc.tile_pool(name="ds_pool", bufs=1, side="right"))
            dense_s = ds_pool.tile([128, NQT, D], F32, name="dense_s")
            sparse_s = ds_pool.tile([128, NQT, D], F32, name="sparse_s")

            # ---- host-replicated row constants (one DMA on the idle
            # sync ring; lands mid-attention) ----
            rows_t = ds_pool.tile([128, 5, D], F32, name="rows_t")
            nc.sync.dma_start(
                out=rows_t,
                in_=rows_pack.ap().rearrange("p (r d) -> p r d", r=5))
            b12_bc = rows_t[:, 1, :]
            g1_bc = rows_t[:, 2, :]
            g2_bc = rows_t[:, 3, :]
            be2_bc = rows_t[:, 4, :]

            xg = ds_pool.tile([128, NQT, D], BF16, name="xg")
            stats = ds_pool.tile([128, NQT, 2, 6], F32, name="stats")
            mv2 = ds_pool.tile([128, NQT, 2], F32, name="mv2")
            sd = ds_pool.tile([128, NQT], F32, name="sd")
            rstd = ds_pool.tile([128, NQT], F32, name="rstd")
            x1s = ds_pool.tile([128, 2, D], F32, name="x1s")
            xhat_bf = ds_pool.tile([128, NQT, D], BF16, name="xhat_bf")

            def ln_normalize(x1, qt, out=None):
                for half in range(2):
                    nc.vector.bn_stats(
                        out=stats[:, qt, half, :],
                        in_=x1[:, half * 512:half * 512 + 512])
                nc.vector.bn_aggr(out=mv2[:, qt, :], in_=stats[:, qt])
                nc.scalar.activation(out=sd[:, qt:qt + 1],
                                     in_=mv2[:, qt, 1:2], func=AF.Sqrt,
                                     bias=eps_t, scale=1.0)
                nc.vector.reciprocal(rstd[:, qt:qt + 1], sd[:, qt:qt + 1])
                nc.vector.tensor_scalar(out=x1 if out is None else out,
                                        in0=x1,
                                        scalar1=mv2[:, qt, 0:1],
                                        scalar2=rstd[:, qt:qt + 1],
                                        op0=OP.subtract, op1=OP.mult)

            xln_stack = contextlib.ExitStack()
            xln_pool = xln_stack.enter_context(
                tc.tile_pool(name="xln_pool", bufs=1, side="right"))
            xlnT = xln_pool.tile([128, KC, SQ], BF16, name="xlnT")

            # ====== per-qt pipeline: out_proj + spmm + fuse + LN1 + xT ======
            with contextlib.ExitStack() as ph5:
                pm_pool = ph5.enter_context(
                    tc.tile_pool(name="pm_pool", bufs=1))
                ps_tr2 = ph5.enter_context(
                    tc.tile_pool(name="ps_tr2", bufs=2, space="PSUM"))
                ps_mm = ph5.enter_context(
                    tc.tile_pool(name="ps_mm", bufs=6, space="PSUM"))
                pmT = pm_pool.tile([128, NTOK, SQ], BF16, name="pmT")
                with nc.named_scope("p5_outproj"):
                    # all masked-p transposes first: they depend only on
                    # psp, so the PE stays busy while the attention tail's
                    # DVE normalize chain drains
                    for qt in range(NQT):
                        for t in range(NTOK):
                            pst = ps_tr2.tile([128, 128], BF16, name="pst2",
                                              tag="pst2")
                            nc.tensor.transpose(
                                pst, psp[qt][:, t * 128:t * 128 + 128],
                                ident_b)
                            nc.scalar.copy(
                                out=pmT[:, t, qt * 128:qt * 128 + 128],
                                in_=pst)
                    for qt in range(NQT):
                        sps = []
                        for nh in range(2):
                            ps = ps_mm.tile([128, 512], F32, name="ps_o",
                                            tag="ps_o")
                            for t in range(NTOK):
                                nc.tensor.matmul(
                                    ps,
                                    pmT[:, t, qt * 128:qt * 128 + 128],
                                    Vsp[:, t, nh * 512:nh * 512 + 512],
                                    start=(t == 0), stop=(t == NTOK - 1))
                            sps.append(ps)
                        # dense out_proj for this query tile
                        pss2 = [ps_mm.tile([128, 512], F32, name="ps_o",
                                           tag="ps_o") for _ in range(2)]
                        for kc in range(KC):
                            for nh in range(2):
                                nc.tensor.matmul(
                                    pss2[nh],
                                    ctxT[:, kc, qt * 128:qt * 128 + 128],
                                    wo_full[:, kc, nh * 512:nh * 512 + 512],
                                    start=(kc == 0), stop=(kc == KC - 1))
                        # fuse directly from PSUM on DVE:
                        #   x1 = sparse_ps*rcp + xot, then += dense_ps*sig
                        x1 = x1s[:, qt % 2, :]
                        for nh in range(2):
                            sl = slice(nh * 512, nh * 512 + 512)
                            nc.vector.scalar_tensor_tensor(
                                out=x1[:, sl], in0=sps[nh],
                                scalar=rcp_sp[:, qt:qt + 1],
                                in1=xot[:, qt, sl],
                                op0=OP.mult, op1=OP.add)
                        for nh in range(2):
                            sl = slice(nh * 512, nh * 512 + 512)
                            nc.vector.scalar_tensor_tensor(
                                out=x1[:, sl], in0=pss2[nh],
                                scalar=sig_bc,
                                in1=x1[:, sl],
                                op0=OP.mult, op1=OP.add)
                        if DBG:
                            for nh in range(2):
                                sl = slice(nh * 512, nh * 512 + 512)
                                nc.scalar.activation(
                                    out=sparse_s[:, qt, sl], in_=sps[nh],
                                    func=AF.Copy, scale=rcp_sp[:, qt:qt + 1])
                                nc.scalar.activation(
                                    out=dense_s[:, qt, sl], in_=pss2[nh],
                                    func=AF.Copy, scale=sig_bc)
                        ln_normalize(x1, qt, out=xhat_bf[:, qt, :])
                        nc.sync.dma_start_transpose(
                            out=xlnT[:, :, qt * 128:qt * 128 + 128],
                            in_=xhat_bf[:, qt, :])
            if DBG:
                for qt in range(NQT):
                    nc.sync.dma_start(
                        out=dbg_dense.ap()[qt * 128:qt * 128 + 128, :],
                        in_=dense_s[:, qt, :])
                    nc.sync.dma_start(
                        out=dbg_sparse.ap()[qt * 128:qt * 128 + 128, :],
                        in_=sparse_s[:, qt, :])
            sp_stack.close()

            # xg (LN2 residual) on DVE while ff1 owns the PE
            for qt in range(NQT):
                nc.vector.tensor_mul(xg[:, qt, :], xhat_bf[:, qt, :], g1_bc)
                nc.vector.tensor_add(xg[:, qt, :], xg[:, qt, :], b12_bc)

            # ============ ff1 + relu ============
            h1_stack = contextlib.ExitStack()
            h1_pool = h1_stack.enter_context(
                tc.tile_pool(name="h1_pool", bufs=1))
            h1T = h1_pool.tile([128, FC, SQ], BF16, name="h1T")
            with contextlib.ExitStack() as ph9:
                w3str = ph9.enter_context(tc.tile_pool(name="w3str", bufs=4))
                ps_f1 = ph9.enter_context(
                    tc.tile_pool(name="ps_f1", bufs=4, space="PSUM"))
                with nc.named_scope("p9_ff1"):
                    for jj in range(16):
                        wt = w3str.tile([128, KC, 256], BF16, name="w1t",
                                        tag="w3")
                        f0 = jj * 256
                        eng = nc.scalar if jj % 2 == 0 else nc.sync
                        eng.dma_start(out=wt, in_=f1T_r[:, :, f0:f0 + 256])
                        for fi in range(2):
                            dft = jj * 2 + fi
                            ps = ps_f1.tile([128, 512], F32, name="ps_f",
                                            tag="ps_f")
                            for kc in range(KC):
                                nc.tensor.matmul(
                                    ps, wt[:, kc, fi * 128:fi * 128 + 128],
                                    xlnT[:, kc, :],
                                    start=(kc == 0), stop=(kc == KC - 1))
                            if jj % 2 == 0:
                                nc.scalar.activation(
                                    out=h1T[:, dft, :], in_=ps,
                                    func=AF.Relu,
                                    bias=b1_c[:, dft:dft + 1], scale=1.0)
                            else:
                                nc.vector.tensor_scalar(
                                    out=h1T[:, dft, :], in0=ps,
                                    scalar1=b1_c[:, dft:dft + 1],
                                    scalar2=0.0, op0=OP.add, op1=OP.max)
            xln_stack.close()

            # ============ ff2 + residual + LN2 + out ============
            ff_s = ds_pool.tile([128, NQT, D], F32, name="ff_s")
            with contextlib.ExitStack() as ph10:
                w4str = ph10.enter_context(tc.tile_pool(name="w4str",
                                                        bufs=8))
                w4b_pool = ph10.enter_context(
                    tc.tile_pool(name="w4b_pool", bufs=1))
                ps_f2 = ph10.enter_context(
                    tc.tile_pool(name="ps_f2", bufs=8, space="PSUM"))
                with nc.named_scope("p10_ff2"):
                    pss = [ps_f2.tile([128, 512], F32, name="ps_g",
                                      tag="ps_g") for _ in range(8)]
                    # prefetch the second column-half of f2T (resident for
                    # the qt-major second pass)
                    f2b = w4b_pool.tile([128, FC, 512], BF16, name="f2b")
                    # pass 0: stream the first half kc-major; the second
                    # half's resident block prefetches 1:1 on the other ring
                    for kc in range(FC):
                        f2h = w4str.tile([128, 512], BF16, name="f2h",
                                         tag="w4")
                        eng = nc.scalar if kc % 2 == 0 else nc.sync
                        eng2 = nc.sync if kc % 2 == 0 else nc.scalar
                        eng.dma_start(out=f2h,
                                      in_=f2T_r[:, kc, 0:512])
                        eng2.dma_start(out=f2b[:, kc, :],
                                       in_=f2T_r[:, kc, 512:1024])
                        for qt in range(NQT):
                            nc.tensor.matmul(
                                pss[2 * qt],
                                h1T[:, kc, qt * 128:qt * 128 + 128],
                                f2h, start=(kc == 0), stop=(kc == FC - 1))
                    for qt in range(NQT):
                        nc.tensor.matmul(
                            pss[2 * qt], ident_b, xg[:, qt, 0:512],
                            start=False, stop=True)
                        nc.vector.bn_stats(out=stats[:, qt, 0, :],
                                           in_=pss[2 * qt])
                    # pass 1: qt-major on the resident half; finish LN2 and
                    # store per qt while later qt's matmuls run
                    for qt in range(NQT):
                        for kc in range(FC):
                            nc.tensor.matmul(
                                pss[2 * qt + 1],
                                h1T[:, kc, qt * 128:qt * 128 + 128],
                                f2b[:, kc, :],
                                start=(kc == 0), stop=(kc == FC - 1))
                        nc.tensor.matmul(
                            pss[2 * qt + 1], ident_b, xg[:, qt, 512:1024],
                            start=False, stop=True)
                        nc.vector.bn_stats(out=stats[:, qt, 1, :],
                                           in_=pss[2 * qt + 1])
                        nc.vector.bn_aggr(out=mv2[:, qt, :],
                                          in_=stats[:, qt])
                        nc.scalar.activation(out=sd[:, qt:qt + 1],
                                             in_=mv2[:, qt, 1:2],
                                             func=AF.Sqrt, bias=eps_t,
                                             scale=1.0)
                        nc.vector.reciprocal(rstd[:, qt:qt + 1],
                                             sd[:, qt:qt + 1])
                        x2 = ff_s[:, qt, :]
                        for half in range(2):
                            nc.vector.tensor_scalar(
                                out=x2[:, half * 512:half * 512 + 512],
                                in0=pss[2 * qt + half],
                                scalar1=mv2[:, qt, 0:1],
                                scalar2=rstd[:, qt:qt + 1],
                                op0=OP.subtract, op1=OP.mult)
                        ot = ds_pool.tile([128, D], F32, name="out_t",
                                          tag="out_t", bufs=2)
                        nc.vector.tensor_mul(ot[:, 0:512], x2[:, 0:512],
                                             g2_bc[:, 0:512])
                        nc.vector.tensor_add(ot[:, 0:512], ot[:, 0:512],
                                             be2_bc[:, 0:512])
                        nc.gpsimd.tensor_mul(ot[:, 512:1024],
                                             x2[:, 512:1024],
                                             g2_bc[:, 512:1024])
                        nc.gpsimd.tensor_add(ot[:, 512:1024],
                                             ot[:, 512:1024],
                                             be2_bc[:, 512:1024])
                        nc.sync.dma_start(
                            out=out.ap()[qt * 128:qt * 128 + 128, :],
                            in_=ot)
            h1_stack.close()
            ds_stack.close()
            ctx_stack.close()
            scr_stack.close()
            psp_stack.close()

    nc.compile()
    return nc


def _prep_inputs(src, in_proj_w, in_proj_b, out_proj_w, out_proj_b,
                 Qp_w, Qp_b, Kp_w, Kp_b, Vp_w, Vp_b, lam,
                 ff1_w, ff1_b, ff2_w, ff2_b, ln1_g, ln1_b, ln2_g, ln2_b):
    import ml_dtypes
    f = np.float32
    A = lambda x: np.ascontiguousarray(x, dtype=f)
    AB = lambda x: np.ascontiguousarray(np.asarray(x, dtype=f),
                                        dtype=ml_dtypes.bfloat16)
    bias_pack = np.zeros((128, 84), np.float32)
    bias_pack[:, 0:24] = np.asarray(in_proj_b, np.float32).reshape(24, 128).T
    bias_pack[:, 24:32] = np.asarray(Vp_b, np.float32).reshape(8, 128).T
    # ln1 affine folded into ff1: relu((x*g1+be1) @ W1.T + b1)
    #   = relu(x @ (W1*g1).T + (b1 + W1@be1))
    w1f = np.asarray(ff1_w, np.float32)
    g1f = np.asarray(ln1_g, np.float32)
    b1_eff = (np.asarray(ff1_b, np.float32)
              + w1f @ np.asarray(ln1_b, np.float32))
    bias_pack[:, 32:64] = b1_eff.reshape(32, 128).T
    bias_pack[:, 64:72] = np.asarray(ln1_g, np.float32).reshape(8, 128).T
    bias_pack[:, 72:80] = np.asarray(ln1_b, np.float32).reshape(8, 128).T
    bias_pack[0:64, 80] = np.asarray(Qp_b, np.float32)
    bias_pack[0:64, 81] = np.asarray(Kp_b, np.float32)
    sigf = 1.0 / (1.0 + np.exp(-np.float32(np.asarray(lam))))
    bias_pack[:, 82] = sigf
    bias_pack[:, 83] = 1.0 - sigf
    # residual bias row: v-projection biases commute through the
    # (normalized) attention rows, so fold them host-side:
    #   sig*(out_proj_b + v_b @ Wo.T) + (1-sig)*Vp_b
    v_b = np.asarray(in_proj_b, np.float32)[2 * D:3 * D]
    bo_row = (sigf * (np.asarray(out_proj_b, np.float32)
                      + v_b @ np.asarray(out_proj_w, np.float32).T)
              + (1.0 - sigf) * np.asarray(Vp_b, np.float32))
    rows = np.stack([
        bo_row,
        np.asarray(ff2_b, np.float32) + np.asarray(ln1_b, np.float32),
        np.asarray(ln1_g, np.float32),
        np.asarray(ln2_g, np.float32),
        np.asarray(ln2_b, np.float32),
    ]).reshape(-1)
    rows_rep = np.ascontiguousarray(
        np.broadcast_to(rows[None, :], (128, 5 * D)), np.float32)
    shared = {
        "wqkvT": AB(np.asarray(in_proj_w).T),
        "woT": AB(np.asarray(out_proj_w).T),
        "vpT": AB(np.asarray(Vp_w).T),
        "qkpT": AB(np.concatenate([np.asarray(Qp_w).T, np.asarray(Kp_w).T],
                                  axis=1)),
        "f1T": AB((w1f * g1f[None, :]).T),
        "f2T": AB(np.asarray(ff2_w).T),
        "bias_pack": A(bias_pack),
        "rows_pack": rows_rep,
    }
    in_maps = []
    for core in range(8):
        b, h = core // 2, core % 2
        srcb = np.asarray(src[b])
        xTb = srcb.T
        if h == 1:
            # own-query columns first (key order is irrelevant to attention)
            xTb = np.concatenate([xTb[:, SQ:], xTb[:, :SQ]], axis=1)
        m = dict(shared)
        m["xT"] = AB(xTb)
        m["x_own"] = A(srcb[h * SQ:(h + 1) * SQ, :] + bo_row[None, :])
        in_maps.append(m)
    return in_maps


def _run(inputs, trace=False):
    if "nc" not in _cached:
        _cached["nc"] = _build()
    nc = _cached["nc"]
    in_maps = _prep_inputs(**inputs)
    res = run_bass_kernel_spmd(nc, in_maps, core_ids=list(range(8)),
                               trace=trace)
    out = np.empty((B, S, D), np.float32)
    for core in range(8):
        b, h = core // 2, core % 2
        out[b, h * SQ:(h + 1) * SQ, :] = res.results[core]["out"]
    return out, res


def kernel(**inputs) -> np.ndarray:
    out, _ = _run(inputs, trace=False)
    return out

